# revision 1
# baseline (speedup 1.0000x reference)
"""Causal self-attention Trainium2 kernel.

Full inputs -> full outputs. Data-parallel over batch across 8 NeuronCores
(16 batches per core), no collectives.

Per-core layout strategy (everything fp32 in SBUF, matmuls run in fp32r):
  - X [tok, C] is PE-transposed to XT [C, tok] (feature-on-partition).
  - Q^T, K^T computed as [feature, tok] (lhsT = w_attn^T tile, rhs = XT),
    bias and the 1/sqrt(hd) scale folded into the PSUM->SBUF eviction.
  - V computed as [tok, feature] (lhsT = XT tile, rhs = w_attn^T V-cols);
    V bias is folded into an effective projection bias on the host.
  - Scores are computed transposed: S^T[k, q] = K^T.T @ Q^T, with the causal
    mask (-1e30) pre-added into PSUM via an identity matmul, then exp (ACT).
  - PV: lhsT = [V | ones] so row 64 of the PSUM output is Z = sum_k P.
  - Normalization: r = 1/Z via a fast custom-DVE reciprocal, broadcast over
    64 partitions with a K=1 matmul, one tensor-tensor multiply -> O^T.
  - Projection: out[tok, C] = O^T.T @ w_proj^T per 128-token tile, effective
    bias added during the PSUM->SBUF eviction.
"""

import numpy as np

import concourse.bass as bass
import concourse.bacc as bacc
import concourse.mybir as mybir
import concourse.tile as tile

N_CORES = 8
B, T, C = 128, 256, 384
H, HD = 6, 64
NB = B // N_CORES          # batches per core
TOK = NB * T               # tokens per core
G = 2                      # batches per group
NG = NB // G               # groups per core
GT = G * T                 # tokens per group (1024)
NTT = GT // 128            # 128-token tiles per group (8)
F32 = mybir.dt.float32
F32R = mybir.dt.float32r
AF = mybir.ActivationFunctionType
NEGBIG = -1.0e30


def _body(tc, x_d, wat_d, wpt_d, bq_d, bk_d, beff_d, mask_d, ident_d, identr_d, ones6_d, y_d):
    nc = tc.nc
    from contextlib import ExitStack

    ctx = ExitStack()
    with ctx:
        const = ctx.enter_context(tc.tile_pool(name="const", bufs=1))
        xin = ctx.enter_context(tc.tile_pool(name="xin", bufs=2))
        xt = ctx.enter_context(tc.tile_pool(name="xt", bufs=2))
        qkt = ctx.enter_context(tc.tile_pool(name="qkt", bufs=2))
        v65 = ctx.enter_context(tc.tile_pool(name="v65", bufs=2))
        pp = ctx.enter_context(tc.tile_pool(name="pp", bufs=4))
        oe = ctx.enter_context(tc.tile_pool(name="oe", bufs=3))
        rp = ctx.enter_context(tc.tile_pool(name="rp", bufs=4))
        dscr = ctx.enter_context(tc.tile_pool(name="dscr", bufs=4, space="DRAM"))
        ot = ctx.enter_context(tc.tile_pool(name="ot", bufs=2))
        yb = ctx.enter_context(tc.tile_pool(name="yb", bufs=3))
        mm_ps = ctx.enter_context(tc.tile_pool(name="mm_ps", bufs=2, space="PSUM"))
        s_ps = ctx.enter_context(tc.tile_pool(name="s_ps", bufs=3, space="PSUM"))
        o_ps = ctx.enter_context(tc.tile_pool(name="o_ps", bufs=3, space="PSUM"))

        dma = nc.sync.dma_start

        wat_sb = const.tile([128, 3, 3 * C], F32R, name="wat_sb")
        wpt_sb = const.tile([64, H, C], F32R, name="wpt_sb")
        bq_sb = const.tile([128, 3], F32, name="bq_sb")
        bk_sb = const.tile([128, 3], F32, name="bk_sb")
        beff_sb = const.tile([128, C], F32, name="beff_sb")
        mask_sb = const.tile([128, 2 * T], F32R, name="mask_sb")
        ident_sb = const.tile([128, 128], F32, name="ident_sb")
        identr_sb = const.tile([128, 128], F32R, name="identr_sb")
        ones6_sb = const.tile([128, H], F32R, name="ones6_sb")

        dma(wat_sb[:], wat_d.ap().rearrange("(ct p) f -> p ct f", p=128))
        dma(wpt_sb[:], wpt_d.ap())
        dma(bq_sb[:], bq_d.ap())
        dma(bk_sb[:], bk_d.ap())
        dma(beff_sb[:], beff_d.ap())
        dma(mask_sb[:], mask_d.ap())
        dma(ident_sb[:], ident_d.ap())
        dma(identr_sb[:], identr_d.ap())
        dma(ones6_sb[:], ones6_d.ap())

        xv = x_d.ap().rearrange("(g tt p) c -> g p tt c", tt=NTT, p=128)
        yv = y_d.ap().rearrange("(g tt p) c -> g tt p c", tt=NTT, p=128)

        for g in range(NG):
            X_sb = xin.tile([128, NTT, C], F32, name=f"X_{g}", tag="X")
            dma(X_sb[:], xv[g])

            # ---- X^T via PE transposes
            XT_sb = xt.tile([128, 3, GT], F32R, name=f"XT_{g}", tag="XT")
            for ct in range(3):
                for half in range(GT // 512):
                    ps = mm_ps.tile([128, 512], F32, name=f"psxt_{g}_{ct}_{half}", tag="mm")
                    for i in range(4):
                        tt = 4 * half + i
                        nc.tensor.transpose(
                            ps[:, 128 * i:128 * (i + 1)],
                            X_sb[:, tt, 128 * ct:128 * (ct + 1)],
                            ident_sb[:],
                        )
                    nc.scalar.copy(XT_sb[:, ct, 512 * half:512 * (half + 1)], ps[:])

            # ---- Q^T / K^T  [feature, tok]
            QKT_sb = qkt.tile([128, 6, GT], F32R, name=f"QKT_{g}", tag="QKT")
            NH = GT // 512
            for ft in range(6):
                pq = [mm_ps.tile([128, 512], F32, name=f"psqk_{g}_{ft}_{i}", tag="mm")
                      for i in range(NH)]
                for ct in range(3):
                    for half in range(NH):
                        nc.tensor.matmul(
                            pq[half][:],
                            wat_sb[:, ct, 128 * ft:128 * (ft + 1)],
                            XT_sb[:, ct, 512 * half:512 * (half + 1)],
                            start=(ct == 0),
                            stop=(ct == 2),
                        )
                for half in range(NH):
                    dst = QKT_sb[:, ft, 512 * half:512 * (half + 1)]
                    if ft < 3:
                        nc.scalar.activation(dst, pq[half][:], AF.Identity,
                                             bias=bq_sb[:, ft:ft + 1], scale=0.125)
                    else:
                        nc.scalar.activation(dst, pq[half][:], AF.Identity,
                                             bias=bk_sb[:, ft - 3:ft - 2], scale=1.0)

            # ---- V [tok, feature] with interleaved ones column
            V65_sb = v65.tile([128, NTT, H * 65], F32R, name=f"V65_{g}", tag="V65")
            for tt in range(NTT):
                psv = mm_ps.tile([128, 384], F32, name=f"psv_{g}_{tt}", tag="mm")
                for ct in range(3):
                    nc.tensor.matmul(
                        psv[:],
                        XT_sb[:, ct, 128 * tt:128 * (tt + 1)],
                        wat_sb[:, ct, 2 * C:3 * C],
                        start=(ct == 0),
                        stop=(ct == 2),
                    )
                v_view = V65_sb[:, tt, :].rearrange("p (h w) -> p h w", h=H)
                nc.vector.tensor_copy(
                    v_view[:, :, 0:64],
                    psv[:].rearrange("p (h w) -> p h w", h=H),
                )
                nc.vector.tensor_copy(v_view[:, :, 64:65],
                                      ones6_sb[:].unsqueeze(2))

            # ---- attention per (batch, head)
            OT_sb = ot.tile([64, H, GT], F32R, name=f"OT_{g}", tag="OT")
            for bl in range(G):
                q0 = 256 * bl
                for hp in range(3):
                    heads = (2 * hp, 2 * hp + 1)
                    ps_list = []
                    # masks first (identity stationary shared)
                    for h in heads:
                        ps_s = s_ps.tile([128, 512], F32, name=f"pss_{g}_{bl}_{h}", tag="s")
                        ps_list.append(ps_s)
                        nc.tensor.matmul(ps_s[:], identr_sb[:],
                                         mask_sb[:], start=True, stop=False)
                    # scores, head pair interleaved (row groups 0/64 overlap)
                    for kt in range(2):
                        for hi, h in enumerate(heads):
                            ft, row0 = h // 2, 64 * (h % 2)
                            KT = QKT_sb[row0:row0 + 64, 3 + ft, :]
                            QT = QKT_sb[row0:row0 + 64, ft, q0:q0 + 256]
                            nc.tensor.matmul(
                                ps_list[hi][:, 256 * kt:256 * (kt + 1)],
                                KT[:, q0 + 128 * kt:q0 + 128 * (kt + 1)],
                                QT,
                                start=False,
                                stop=(kt == 1),
                            )
                    for hi, h in enumerate(heads):
                        P_sb = pp.tile([128, 512], F32R, name=f"P_{g}_{bl}_{h}", tag="P")
                        nc.scalar.activation(P_sb[:], ps_list[hi][:], AF.Exp)
                        ps_o = o_ps.tile([128, 256], F32, name=f"pso_{g}_{bl}_{h}", tag="o")
                        nc.tensor.matmul(ps_o[0:65, :],
                                         V65_sb[:, 2 * bl, 65 * h:65 * h + 65],
                                         P_sb[:, 0:256], start=True, stop=False)
                        nc.tensor.matmul(ps_o[0:65, :],
                                         V65_sb[:, 2 * bl + 1, 65 * h:65 * h + 65],
                                         P_sb[:, 256:512], start=False, stop=True)
                        Oe_sb = oe.tile([128, 256], F32, name=f"Oe_{g}_{bl}_{h}", tag="Oe")
                        nc.scalar.copy(Oe_sb[0:65, :], ps_o[0:65, :])
                        zscr = dscr.tile([1, 256], F32, name=f"zs_{g}_{bl}_{h}", tag="zs")
                        rscr = dscr.tile([1, 256], F32, name=f"rs_{g}_{bl}_{h}", tag="rs")
                        zt_sb = rp.tile([128, 2], F32, name=f"zt_{g}_{bl}_{h}", tag="zt")
                        rt_sb = rp.tile([128, 2], F32, name=f"rt_{g}_{bl}_{h}", tag="rt")
                        rbc_sb = rp.tile([64, 256], F32, name=f"rbc_{g}_{bl}_{h}", tag="rbc")
                        dma(zscr[:], Oe_sb[64:65, :])
                        dma(zt_sb[:], zscr[:].rearrange("o (p f) -> (o p) f", p=128))
                        nc.vector.reciprocal(rt_sb[:], zt_sb[:])
                        dma(rscr[:].rearrange("o (p f) -> (o p) f", p=128), rt_sb[:])
                        dma(rbc_sb[:], rscr[:].broadcast_to([64, 256]))
                        nc.vector.tensor_mul(OT_sb[:, h, q0:q0 + 256],
                                             Oe_sb[0:64, :], rbc_sb[:])

            # ---- projection [tok, C]
            for tt in range(NTT):
                ps_y = mm_ps.tile([128, 384], F32, name=f"psy_{g}_{tt}", tag="mm")
                for hh in range(H):
                    nc.tensor.matmul(
                        ps_y[:],
                        OT_sb[:, hh, 128 * tt:128 * (tt + 1)],
                        wpt_sb[:, hh, :],
                        start=(hh == 0),
                        stop=(hh == H - 1),
                    )
                Y_sb = yb.tile([128, C], F32, name=f"Y_{g}_{tt}", tag="Y")
                nc.vector.tensor_add(Y_sb[:], ps_y[:], beff_sb[:])
                dma(yv[g, tt], Y_sb[:])


_CACHE = {}


def _build_nc():
    if "nc" in _CACHE:
        return _CACHE["nc"]
    nc = bacc.Bacc("TRN2", target_bir_lowering=False, debug=False,
                   num_devices=N_CORES)
    x_d = nc.dram_tensor("x", [TOK, C], F32, kind="ExternalInput")
    wat_d = nc.dram_tensor("w_attnT", [C, 3 * C], F32R, kind="ExternalInput")
    wpt_d = nc.dram_tensor("w_projT", [64, H, C], F32R, kind="ExternalInput")
    bq_d = nc.dram_tensor("bq", [128, 3], F32, kind="ExternalInput")
    bk_d = nc.dram_tensor("bk", [128, 3], F32, kind="ExternalInput")
    beff_d = nc.dram_tensor("beff", [128, C], F32, kind="ExternalInput")
    mask_d = nc.dram_tensor("maskS", [128, 2 * T], F32R, kind="ExternalInput")
    ident_d = nc.dram_tensor("ident", [128, 128], F32, kind="ExternalInput")
    identr_d = nc.dram_tensor("identr", [128, 128], F32R, kind="ExternalInput")
    ones6_d = nc.dram_tensor("ones6", [128, H], F32R, kind="ExternalInput")
    y_d = nc.dram_tensor("y", [TOK, C], F32, kind="ExternalOutput")

    with tile.TileContext(nc) as tc:
        _body(tc, x_d, wat_d, wpt_d, bq_d, bk_d, beff_d, mask_d, ident_d,
              identr_d, ones6_d, y_d)
    nc.compile()
    _CACHE["nc"] = nc
    return nc


def _host_inputs(x, w_attn, b_attn, w_proj, b_proj):
    """Build the per-core input maps (host-side prep of weights/constants)."""
    w_attnT = np.ascontiguousarray(w_attn.T)                       # [C, 3C]
    # w_projT regrouped per head: [64, H, C]; wpt[p, h, of] = w_proj[of, 64h+p]
    wpt = np.ascontiguousarray(w_proj.T.reshape(H, 64, C).transpose(1, 0, 2))
    bq = np.ascontiguousarray((0.125 * b_attn[:C]).reshape(3, 128).T)
    bk = np.ascontiguousarray(b_attn[C:2 * C].reshape(3, 128).T)
    b_eff = b_proj + w_proj @ b_attn[2 * C:]
    beff = np.ascontiguousarray(np.broadcast_to(b_eff, (128, C))).astype(np.float32)

    # mask for S^T bank [128, 512]: cols j<256: (k=p, q=j); cols j>=256:
    # (k=128+p, q=j-256)
    p = np.arange(128)[:, None]
    j = np.arange(512)[None, :]
    valid = np.where(j < 256, p <= j, p <= j - 384)
    mask = np.where(valid, 0.0, NEGBIG).astype(np.float32)

    ident = np.eye(128, dtype=np.float32)

    common = {
        "w_attnT": w_attnT.astype(np.float32),
        "w_projT": wpt.astype(np.float32),
        "bq": bq.astype(np.float32),
        "bk": bk.astype(np.float32),
        "beff": beff,
        "maskS": mask,
        "ident": ident,
        "identr": ident,
        "ones6": np.ones((128, H), dtype=np.float32),
    }
    xs = x.reshape(N_CORES, TOK, C)
    in_maps = []
    for c in range(N_CORES):
        m = dict(common)
        m["x"] = np.ascontiguousarray(xs[c]).astype(np.float32)
        in_maps.append(m)
    return in_maps


def kernel(x, w_attn, b_attn, w_proj, b_proj):
    from concourse.bass_utils import run_bass_kernel_spmd

    x = np.asarray(x, dtype=np.float32)
    w_attn = np.asarray(w_attn, dtype=np.float32)
    b_attn = np.asarray(b_attn, dtype=np.float32)
    w_proj = np.asarray(w_proj, dtype=np.float32)
    b_proj = np.asarray(b_proj, dtype=np.float32)

    nc = _build_nc()
    in_maps = _host_inputs(x, w_attn, b_attn, w_proj, b_proj)
    res = run_bass_kernel_spmd(nc, in_maps, core_ids=list(range(N_CORES)))
    y = np.stack([res.results[c]["y"] for c in range(N_CORES)])
    return y.reshape(B, T, C)



# revision 5
# speedup vs baseline: 2.8142x; 2.8142x over previous
"""Causal self-attention Trainium2 kernel (v2).

Full inputs -> full outputs. Data-parallel over batch across 8 NeuronCores
(16 batches per core), no collectives.

Per-core design (all matmul operands bf16, PSUM accumulation fp32):
  - x is cast to bf16 on the host; X^T tiles land in SBUF feature-major via
    xbar transposed DMA (no PE transposes, no eviction traffic).
  - Q^T/K^T [feat, tok] from stationary weight tiles; 1/sqrt(hd) is folded
    into the Q weights/bias on the host. Bias added during PSUM eviction
    (alternating ACT/DVE to balance engines).
  - V [tok, feat] with an interleaved ones column per head (V65) so the
    PV matmul also produces the softmax denominator Z.
  - Scores are computed causally: per (batch,head) only blocks
    (k0,q0),(k0,q1),(k1,q1); the (k1,q0) block is skipped. The causal mask
    is pre-added into PSUM for the two diagonal blocks only via N=128
    identity matmuls. Heads of a pair run in distinct 64-partition row
    groups so their K=64 score matmuls execute concurrently on the PE.
  - P^T = exp(S^T) evicted to SBUF as bf16 (single ACT op per head).
  - PV is q-major: lhsT = P^T block (stationary, 128 cols -> fast weight
    load), rhs = V65 slice (N=65). Output O[q, 64]+Z lands with queries on
    partitions, so normalization is a per-partition scale: one DVE
    reciprocal per head-pair (strided [128,4] view of Z columns) and one
    scale-by-vector eviction per (head, qtile), split across ACT and DVE.
  - O tiles are PE-transposed (bf16, stays in PSUM as bf16) and evicted to
    OGT [feat, tok]; projection runs 3 full-K=128 matmuls per token tile.
    The output bias is pre-loaded into PSUM with a K=1 ones x beff matmul;
    Y is evicted by DVE and DMAed out.
"""

import numpy as np

import concourse.bass as bass
import concourse.bacc as bacc
import concourse.mybir as mybir
import concourse.tile as tile

N_CORES = 8
B, T, C = 128, 256, 384
H, HD = 6, 64
NB = B // N_CORES          # batches per core
TOK = NB * T               # tokens per core
G = 2                      # batches per group
NG = NB // G               # groups per core
GT = G * T                 # tokens per group (512)
NTT = GT // 128            # 128-token tiles per group (4)
F32 = mybir.dt.float32
F32R = mybir.dt.float32r
BF16 = mybir.dt.bfloat16
AF = mybir.ActivationFunctionType
NEGBIG = -1.0e30


def _body(tc, x_d, wqk_d, wv_d, wp_d, bqk_d, beffr_d, onecol_d, identb_d,
          trimask_d, y_d):
    nc = tc.nc
    from contextlib import ExitStack

    ctx = ExitStack()
    with ctx:
        const = ctx.enter_context(tc.tile_pool(name="const", bufs=1))
        xt = ctx.enter_context(tc.tile_pool(name="xt", bufs=2))
        qkt = ctx.enter_context(tc.tile_pool(name="qkt", bufs=2))
        v65p = ctx.enter_context(tc.tile_pool(name="v65p", bufs=2))
        pp = ctx.enter_context(tc.tile_pool(name="pp", bufs=3))
        ogp = ctx.enter_context(tc.tile_pool(name="ogp", bufs=2))
        ogtp = ctx.enter_context(tc.tile_pool(name="ogtp", bufs=2))
        rp = ctx.enter_context(tc.tile_pool(name="rp", bufs=4))
        yb = ctx.enter_context(tc.tile_pool(name="yb", bufs=3))
        mm_ps = ctx.enter_context(tc.tile_pool(name="mm_ps", bufs=2, space="PSUM"))
        s_ps = ctx.enter_context(tc.tile_pool(name="s_ps", bufs=2, space="PSUM"))
        pv_ps = ctx.enter_context(tc.tile_pool(name="pv_ps", bufs=2, space="PSUM"))
        t_ps = ctx.enter_context(tc.tile_pool(name="t_ps", bufs=2, space="PSUM"))

        dma = nc.sync.dma_start

        wqk_sb = const.tile([128, 3, H, 128], BF16, name="wqk_sb")
        wv_sb = const.tile([128, 3, C], BF16, name="wv_sb")
        wp_sb = const.tile([128, 3, C], BF16, name="wp_sb")
        bqk_sb = const.tile([128, H], F32, name="bqk_sb")
        beffr_sb = const.tile([1, C], F32R, name="beffr_sb")
        onecol_sb = const.tile([1, 128], F32R, name="onecol_sb")
        identb_sb = const.tile([128, 128], BF16, name="identb_sb")
        trimask_sb = const.tile([128, 128], BF16, name="trimask_sb")
        ones_sb = const.tile([128, NTT * H], BF16, name="ones_sb")

        dma(wqk_sb[:], wqk_d.ap().rearrange("p (a h f) -> p a h f", a=3, h=H))
        dma(wv_sb[:], wv_d.ap().rearrange("p (a f) -> p a f", a=3))
        dma(wp_sb[:], wp_d.ap().rearrange("p (a f) -> p a f", a=3))
        dma(bqk_sb[:], bqk_d.ap())
        dma(beffr_sb[:], beffr_d.ap())
        dma(onecol_sb[:], onecol_d.ap())
        dma(identb_sb[:], identb_d.ap())
        dma(trimask_sb[:], trimask_d.ap())
        nc.vector.memset(ones_sb[:], 1.0)

        yv = y_d.ap().rearrange("(g tt p) c -> g tt p c", tt=NTT, p=128)

        for g in range(NG):
            # ---- X^T via xbar transposed DMA (feature-major, bf16)
            XT = xt.tile([128, 3, GT], BF16, name=f"XT_{g}", tag="XT")
            for ct in range(3):
                nc.sync.dma_start_transpose(
                    XT[:, ct, :],
                    x_d.ap()[g * GT:(g + 1) * GT, 128 * ct:128 * (ct + 1)],
                )

            # ---- Q^T / K^T  [feature, tok]; Q rows pre-scaled by 1/8
            QKT = qkt.tile([128, H, GT], BF16, name=f"QKT_{g}", tag="QKT")
            for ft in range(H):
                ps = mm_ps.tile([128, GT], F32, name=f"psqk_{g}_{ft}", tag="mm")
                for ct in range(3):
                    nc.tensor.matmul(
                        ps[:],
                        wqk_sb[:, ct, ft, :],
                        XT[:, ct, :],
                        start=(ct == 0),
                        stop=(ct == 2),
                    )
                if ft % 2 == 0:
                    nc.scalar.activation(QKT[:, ft, :], ps[:], AF.Identity,
                                         bias=bqk_sb[:, ft:ft + 1])
                else:
                    nc.vector.tensor_scalar_add(QKT[:, ft, :], ps[:],
                                                bqk_sb[:, ft:ft + 1])

            # ---- V [tok, feat] + ones column per head (V65)
            V65 = v65p.tile([128, NTT, H, 65], BF16, name=f"V65_{g}", tag="V65")
            nc.vector.tensor_copy(
                V65[:, :, :, 64],
                ones_sb[:].rearrange("p (a h) -> p a h", a=NTT),
            )
            for tt in range(NTT):
                psv = mm_ps.tile([128, C], F32, name=f"psv_{g}_{tt}", tag="mm")
                for ct in range(3):
                    nc.tensor.matmul(
                        psv[:],
                        XT[:, ct, 128 * tt:128 * (tt + 1)],
                        wv_sb[:, ct, :],
                        start=(ct == 0),
                        stop=(ct == 2),
                    )
                nc.vector.tensor_copy(
                    V65[:, tt, :, 0:64],
                    psv[:].rearrange("p (h w) -> p h w", h=H),
                )

            # ---- attention per (batch, head-pair); causal blocks only
            OG = ogp.tile([128, NTT, C], BF16, name=f"OG_{g}", tag="OG")
            for bl in range(G):
                q0 = 256 * bl
                for hp in range(3):
                    pvps = pv_ps.tile([128, 260], F32, name=f"pv_{g}_{bl}_{hp}",
                                      tag="pv")
                    for hi in range(2):
                        h = 2 * hp + hi
                        r0 = 64 * (h % 2)
                        ftq, ftk = h // 2, 3 + h // 2
                        QT = QKT[r0:r0 + 64, ftq, :]
                        KT = QKT[r0:r0 + 64, ftk, :]
                        sps = s_ps.tile([128, 384], F32, name=f"s_{g}_{bl}_{h}",
                                        tag="s")
                        # One accumulation group per bank: first matmul
                        # start=True arms the whole bank pending-zero;
                        # later matmuls overwrite on first touch of their
                        # columns, accumulate after.
                        # masks into diagonal blocks (cols 0-127 & 256-383)
                        nc.tensor.matmul(sps[:, 0:128], identb_sb[:],
                                         trimask_sb[:], start=True, stop=False)
                        nc.tensor.matmul(sps[:, 256:384], identb_sb[:],
                                         trimask_sb[:], start=False, stop=False)
                        # scores: (k0,q0) (k0,q1) (k1,q1)
                        nc.tensor.matmul(
                            sps[:, 0:128],
                            KT[:, q0:q0 + 128], QT[:, q0:q0 + 128],
                            start=False, stop=False)
                        nc.tensor.matmul(
                            sps[:, 128:256],
                            KT[:, q0:q0 + 128], QT[:, q0 + 128:q0 + 256],
                            start=False, stop=False)
                        nc.tensor.matmul(
                            sps[:, 256:384],
                            KT[:, q0 + 128:q0 + 256], QT[:, q0 + 128:q0 + 256],
                            start=False, stop=True)
                        P = pp.tile([128, 384], BF16, name=f"P_{g}_{bl}_{h}",
                                    tag="P")
                        nc.scalar.activation(P[:], sps[:], AF.Exp)
                        # PV q-major: lhsT = P^T block, rhs = V65 slice.
                        # Single accumulation group for the shared bank.
                        c0 = 130 * hi
                        nc.tensor.matmul(
                            pvps[:, c0:c0 + 65],
                            P[:, 0:128], V65[:, 2 * bl, h, :],
                            start=(hi == 0), stop=False)
                        nc.tensor.matmul(
                            pvps[:, c0 + 65:c0 + 130],
                            P[:, 128:256], V65[:, 2 * bl, h, :],
                            start=False, stop=False)
                        nc.tensor.matmul(
                            pvps[:, c0 + 65:c0 + 130],
                            P[:, 256:384], V65[:, 2 * bl + 1, h, :],
                            start=False, stop=(hi == 1))
                    # 1/Z for both heads & qtiles: strided [128,4] view
                    rt = rp.tile([128, 4], F32, name=f"rt_{g}_{bl}_{hp}",
                                 tag="rt")
                    zview = pvps[:].rearrange("p (a c) -> p a c", c=65)[:, :, 64]
                    nc.vector.reciprocal(rt[:], zview)
                    for hi in range(2):
                        h = 2 * hp + hi
                        for qt in range(2):
                            src = pvps[:, 130 * hi + 65 * qt:
                                       130 * hi + 65 * qt + 64]
                            dst = OG[:, 2 * bl + qt, 64 * h:64 * h + 64]
                            sc = rt[:, 2 * hi + qt:2 * hi + qt + 1]
                            if qt == 0:
                                nc.scalar.mul(dst, src, sc)
                            else:
                                nc.vector.tensor_scalar_mul(dst, src, sc)

            # ---- O^T via PE transposes (bf16 stays bf16 in PSUM)
            OGT = ogtp.tile([128, 3, GT], BF16, name=f"OGT_{g}", tag="OGT")
            for ct in range(3):
                tps = t_ps.tile([128, GT], BF16, name=f"t_{g}_{ct}", tag="t")
                for tt in range(NTT):
                    nc.tensor.transpose(
                        tps[:, 128 * tt:128 * (tt + 1)],
                        OG[:, tt, 128 * ct:128 * (ct + 1)],
                        identb_sb[:],
                    )
                nc.vector.tensor_copy(OGT[:, ct, :], tps[:])

            # ---- projection [tok, C]; bias pre-loaded via K=1 matmul
            for tt in range(NTT):
                yps = mm_ps.tile([128, C], F32, name=f"y_{g}_{tt}", tag="mm")
                nc.tensor.matmul(yps[:], onecol_sb[:], beffr_sb[:],
                                 start=True, stop=False)
                for ct in range(3):
                    nc.tensor.matmul(
                        yps[:],
                        OGT[:, ct, 128 * tt:128 * (tt + 1)],
                        wp_sb[:, ct, :],
                        start=False,
                        stop=(ct == 2),
                    )
                Y = yb.tile([128, C], F32, name=f"Y_{g}_{tt}", tag="Y")
                nc.vector.tensor_copy(Y[:], yps[:])
                dma(yv[g, tt], Y[:])


_CACHE = {}


def _build_nc():
    if "nc" in _CACHE:
        return _CACHE["nc"]
    nc = bacc.Bacc("TRN2", target_bir_lowering=False, debug=False,
                   num_devices=N_CORES)
    x_d = nc.dram_tensor("x", [TOK, C], BF16, kind="ExternalInput")
    wqk_d = nc.dram_tensor("wqk", [128, 3 * H * 128], BF16, kind="ExternalInput")
    wv_d = nc.dram_tensor("wv", [128, 3 * C], BF16, kind="ExternalInput")
    wp_d = nc.dram_tensor("wp", [128, 3 * C], BF16, kind="ExternalInput")
    bqk_d = nc.dram_tensor("bqk", [128, H], F32, kind="ExternalInput")
    beffr_d = nc.dram_tensor("beffr", [1, C], F32R, kind="ExternalInput")
    onecol_d = nc.dram_tensor("onecol", [1, 128], F32R, kind="ExternalInput")
    identb_d = nc.dram_tensor("identb", [128, 128], BF16, kind="ExternalInput")
    trimask_d = nc.dram_tensor("trimask", [128, 128], BF16, kind="ExternalInput")
    y_d = nc.dram_tensor("y", [TOK, C], F32, kind="ExternalOutput")

    with tile.TileContext(nc) as tc:
        _body(tc, x_d, wqk_d, wv_d, wp_d, bqk_d, beffr_d, onecol_d, identb_d,
              trimask_d, y_d)
    nc.compile()
    _CACHE["nc"] = nc
    return nc


def _host_inputs(x, w_attn, b_attn, w_proj, b_proj):
    """Per-core input maps; weights pre-transposed/cast on the host."""
    import ml_dtypes

    bf16 = ml_dtypes.bfloat16
    ws = np.array(w_attn, dtype=np.float32).copy()
    bs = np.array(b_attn, dtype=np.float32).copy()
    ws[:C] *= 0.125        # fold 1/sqrt(hd) into Q
    bs[:C] *= 0.125

    # wqk[k, ct, ft, m] = ws[128*ft + m, 128*ct + k]
    wqk = ws[:2 * C].reshape(H, 128, 3, 128).transpose(3, 2, 0, 1)
    wqk = np.ascontiguousarray(wqk).astype(bf16).reshape(128, 3 * H * 128)
    # wv[k, ct, n] = w_attn[2C + n, 128*ct + k]
    wv = ws[2 * C:].reshape(C, 3, 128).transpose(2, 1, 0)
    wv = np.ascontiguousarray(wv).astype(bf16).reshape(128, 3 * C)
    # wp[k, ct, n] = w_proj[n, 128*ct + k]
    wp = np.array(w_proj, np.float32).reshape(C, 3, 128).transpose(2, 1, 0)
    wp = np.ascontiguousarray(wp).astype(bf16).reshape(128, 3 * C)

    bqk = np.ascontiguousarray(bs[:2 * C].reshape(H, 128).T).astype(np.float32)
    beffr = (b_proj + w_proj @ b_attn[2 * C:]).astype(np.float32).reshape(1, C)
    onecol = np.ones((1, 128), dtype=np.float32)
    ident = np.eye(128, dtype=np.float32).astype(bf16)

    p = np.arange(128)[:, None]
    j = np.arange(128)[None, :]
    trimask = np.where(p > j, NEGBIG, 0.0).astype(np.float32).astype(bf16)

    common = {
        "wqk": wqk, "wv": wv, "wp": wp, "bqk": bqk, "beffr": beffr,
        "onecol": onecol, "identb": ident, "trimask": trimask,
    }
    xs = np.array(x, np.float32).reshape(N_CORES, TOK, C)
    in_maps = []
    for c in range(N_CORES):
        m = dict(common)
        m["x"] = np.ascontiguousarray(xs[c]).astype(bf16)
        in_maps.append(m)
    return in_maps


def kernel(x, w_attn, b_attn, w_proj, b_proj):
    from concourse.bass_utils import run_bass_kernel_spmd

    x = np.asarray(x, dtype=np.float32)
    w_attn = np.asarray(w_attn, dtype=np.float32)
    b_attn = np.asarray(b_attn, dtype=np.float32)
    w_proj = np.asarray(w_proj, dtype=np.float32)
    b_proj = np.asarray(b_proj, dtype=np.float32)

    nc = _build_nc()
    in_maps = _host_inputs(x, w_attn, b_attn, w_proj, b_proj)
    res = run_bass_kernel_spmd(nc, in_maps, core_ids=list(range(N_CORES)))
    y = np.stack([res.results[c]["y"] for c in range(N_CORES)])
    return y.reshape(B, T, C)


# revision 10
# speedup vs baseline: 2.8544x; 1.0143x over previous
"""Causal self-attention Trainium2 kernel (v2).

Full inputs -> full outputs. Data-parallel over batch across 8 NeuronCores
(16 batches per core), no collectives.

Per-core design (all matmul operands bf16, PSUM accumulation fp32):
  - x is cast to bf16 on the host; X^T tiles land in SBUF feature-major via
    xbar transposed DMA (no PE transposes, no eviction traffic).
  - Q^T/K^T [feat, tok] from stationary weight tiles; 1/sqrt(hd) is folded
    into the Q weights/bias on the host. Bias added during PSUM eviction
    (alternating ACT/DVE to balance engines).
  - V [tok, feat] with an interleaved ones column per head (V65) so the
    PV matmul also produces the softmax denominator Z.
  - Scores are computed causally: per (batch,head) only blocks
    (k0,q0),(k0,q1),(k1,q1); the (k1,q0) block is skipped. The causal mask
    is pre-added into PSUM for the two diagonal blocks only via N=128
    identity matmuls. Heads of a pair run in distinct 64-partition row
    groups so their K=64 score matmuls execute concurrently on the PE.
  - P^T = exp(S^T) evicted to SBUF as bf16 (single ACT op per head).
  - PV is q-major: lhsT = P^T block (stationary, 128 cols -> fast weight
    load), rhs = V65 slice (N=65). Output O[q, 64]+Z lands with queries on
    partitions, so normalization is a per-partition scale: one DVE
    reciprocal per head-pair (strided [128,4] view of Z columns) and one
    scale-by-vector eviction per (head, qtile), split across ACT and DVE.
  - O tiles are PE-transposed (bf16, stays in PSUM as bf16) and evicted to
    OGT [feat, tok]; projection runs 3 full-K=128 matmuls per token tile.
    The output bias is pre-loaded into PSUM with a K=1 ones x beff matmul;
    Y is evicted by DVE and DMAed out.
"""

import numpy as np

import concourse.bass as bass
import concourse.bacc as bacc
import concourse.mybir as mybir
import concourse.tile as tile

N_CORES = 8
B, T, C = 128, 256, 384
H, HD = 6, 64
NB = B // N_CORES          # batches per core
TOK = NB * T               # tokens per core
G = 2                      # batches per group
NG = NB // G               # groups per core
GT = G * T                 # tokens per group (512)
NTT = GT // 128            # 128-token tiles per group (4)
F32 = mybir.dt.float32
F32R = mybir.dt.float32r
BF16 = mybir.dt.bfloat16
AF = mybir.ActivationFunctionType
NEGBIG = -1.0e30


def _body(tc, x_d, wqk_d, wv_d, wp_d, bqk_d, beffr_d, onecol_d, identb_d,
          trimask_d, y_d):
    nc = tc.nc
    from contextlib import ExitStack

    ctx = ExitStack()
    with ctx:
        const = ctx.enter_context(tc.tile_pool(name="const", bufs=1))
        xt = ctx.enter_context(tc.tile_pool(name="xt", bufs=2))
        qkt = ctx.enter_context(tc.tile_pool(name="qkt", bufs=2))
        v65p = ctx.enter_context(tc.tile_pool(name="v65p", bufs=2))
        pp = ctx.enter_context(tc.tile_pool(name="pp", bufs=3))
        ogp = ctx.enter_context(tc.tile_pool(name="ogp", bufs=2))
        ogtp = ctx.enter_context(tc.tile_pool(name="ogtp", bufs=2))
        rp = ctx.enter_context(tc.tile_pool(name="rp", bufs=4))
        yb = ctx.enter_context(tc.tile_pool(name="yb", bufs=3))
        mm_ps = ctx.enter_context(tc.tile_pool(name="mm_ps", bufs=2, space="PSUM"))
        s_ps = ctx.enter_context(tc.tile_pool(name="s_ps", bufs=2, space="PSUM"))
        pv_ps = ctx.enter_context(tc.tile_pool(name="pv_ps", bufs=2, space="PSUM"))
        t_ps = ctx.enter_context(tc.tile_pool(name="t_ps", bufs=2, space="PSUM"))

        dma = nc.sync.dma_start

        wqk_sb = const.tile([128, 3, H, 128], BF16, name="wqk_sb")
        wv_sb = const.tile([128, 3, C], BF16, name="wv_sb")
        wp_sb = const.tile([128, 3, C], BF16, name="wp_sb")
        bqk_sb = const.tile([128, H], F32, name="bqk_sb")
        beffr_sb = const.tile([1, C], F32R, name="beffr_sb")
        onecol_sb = const.tile([1, 128], F32R, name="onecol_sb")
        identb_sb = const.tile([128, 128], BF16, name="identb_sb")
        trimask_sb = const.tile([128, 256], BF16, name="trimask_sb")
        ones_sb = const.tile([128, NTT * H], BF16, name="ones_sb")

        dma(wqk_sb[:], wqk_d.ap().rearrange("p (a h f) -> p a h f", a=3, h=H))
        dma(wv_sb[:], wv_d.ap().rearrange("p (a f) -> p a f", a=3))
        dma(wp_sb[:], wp_d.ap().rearrange("p (a f) -> p a f", a=3))
        dma(bqk_sb[:], bqk_d.ap())
        dma(beffr_sb[:], beffr_d.ap())
        dma(onecol_sb[:], onecol_d.ap())
        dma(identb_sb[:], identb_d.ap())
        dma(trimask_sb[:], trimask_d.ap())
        nc.vector.memset(ones_sb[:], 1.0)

        yv = y_d.ap().rearrange("(g tt p) c -> g tt p c", tt=NTT, p=128)

        def qkv_stage(g):
            # ---- X^T via xbar transposed DMA (feature-major, bf16)
            XT = xt.tile([128, 3, GT], BF16, name=f"XT_{g}", tag="XT")
            for ct in range(3):
                nc.sync.dma_start_transpose(
                    XT[:, ct, :],
                    x_d.ap()[g * GT:(g + 1) * GT, 128 * ct:128 * (ct + 1)],
                )

            # ---- Q^T / K^T  [feature, tok]; Q rows pre-scaled by 1/8
            QKT = qkt.tile([128, H, GT], BF16, name=f"QKT_{g}", tag="QKT")
            for ft in range(H):
                ps = mm_ps.tile([128, GT], F32, name=f"psqk_{g}_{ft}", tag="mm")
                for ct in range(3):
                    nc.tensor.matmul(
                        ps[:],
                        wqk_sb[:, ct, ft, :],
                        XT[:, ct, :],
                        start=(ct == 0),
                        stop=(ct == 2),
                    )
                if ft % 2 == 0:
                    nc.scalar.activation(QKT[:, ft, :], ps[:], AF.Identity,
                                         bias=bqk_sb[:, ft:ft + 1])
                else:
                    nc.vector.tensor_scalar_add(QKT[:, ft, :], ps[:],
                                                bqk_sb[:, ft:ft + 1])

            # ---- V [tok, feat] + ones column per head (V65)
            V65 = v65p.tile([128, NTT, H, 65], BF16, name=f"V65_{g}", tag="V65")
            nc.vector.tensor_copy(
                V65[:, :, :, 64],
                ones_sb[:].rearrange("p (a h) -> p a h", a=NTT),
            )
            for tt in range(NTT):
                psv = mm_ps.tile([128, C], F32, name=f"psv_{g}_{tt}", tag="mm")
                for ct in range(3):
                    nc.tensor.matmul(
                        psv[:],
                        XT[:, ct, 128 * tt:128 * (tt + 1)],
                        wv_sb[:, ct, :],
                        start=(ct == 0),
                        stop=(ct == 2),
                    )
                nc.vector.tensor_copy(
                    V65[:, tt, :, 0:64],
                    psv[:].rearrange("p (h w) -> p h w", h=H),
                )
            return QKT, V65

        def attn_stage(g, QKT, V65):
            # ---- attention per (batch, head-pair); causal blocks only
            OG = ogp.tile([128, NTT, C], BF16, name=f"OG_{g}", tag="OG")
            for bl in range(G):
                q0 = 256 * bl
                for hp in range(3):
                    pvps = pv_ps.tile([128, 260], F32, name=f"pv_{g}_{bl}_{hp}",
                                      tag="pv")
                    for hi in range(2):
                        h = 2 * hp + hi
                        r0 = 64 * (h % 2)
                        ftq, ftk = h // 2, 3 + h // 2
                        QT = QKT[r0:r0 + 64, ftq, :]
                        KT = QKT[r0:r0 + 64, ftk, :]
                        sps = s_ps.tile([128, 384], F32, name=f"s_{g}_{bl}_{h}",
                                        tag="s")
                        # One accumulation group per bank: first matmul
                        # start=True arms the whole bank pending-zero;
                        # later matmuls overwrite on first touch of their
                        # columns, accumulate after.
                        # masks into the two diagonal blocks
                        nc.tensor.matmul(
                            sps[:, 0:128], identb_sb[:], trimask_sb[:, 0:128],
                            start=True, stop=False)
                        nc.tensor.matmul(
                            sps[:, 256:384], identb_sb[:], trimask_sb[:, 128:256],
                            start=False, stop=False)
                        # scores: (k0,q0) (k0,q1) (k1,q1)
                        nc.tensor.matmul(
                            sps[:, 0:128],
                            KT[:, q0:q0 + 128], QT[:, q0:q0 + 128],
                            start=False, stop=False)
                        nc.tensor.matmul(
                            sps[:, 128:256],
                            KT[:, q0:q0 + 128], QT[:, q0 + 128:q0 + 256],
                            start=False, stop=False)
                        nc.tensor.matmul(
                            sps[:, 256:384],
                            KT[:, q0 + 128:q0 + 256], QT[:, q0 + 128:q0 + 256],
                            start=False, stop=True)
                        P = pp.tile([128, 384], BF16, name=f"P_{g}_{bl}_{h}",
                                    tag="P")
                        nc.scalar.activation(P[:], sps[:], AF.Exp)
                        # PV q-major: lhsT = P^T block, rhs = V65 slice.
                        # Single accumulation group for the shared bank.
                        c0 = 130 * hi
                        nc.tensor.matmul(
                            pvps[:, c0:c0 + 65],
                            P[:, 0:128], V65[:, 2 * bl, h, :],
                            start=(hi == 0), stop=False)
                        nc.tensor.matmul(
                            pvps[:, c0 + 65:c0 + 130],
                            P[:, 128:256], V65[:, 2 * bl, h, :],
                            start=False, stop=False)
                        nc.tensor.matmul(
                            pvps[:, c0 + 65:c0 + 130],
                            P[:, 256:384], V65[:, 2 * bl + 1, h, :],
                            start=False, stop=(hi == 1))
                    # 1/Z for both heads & qtiles: strided [128,4] view
                    rt = rp.tile([128, 4], F32, name=f"rt_{g}_{bl}_{hp}",
                                 tag="rt")
                    zview = pvps[:].rearrange("p (a c) -> p a c", c=65)[:, :, 64]
                    nc.vector.reciprocal(rt[:], zview)
                    for hi in range(2):
                        h = 2 * hp + hi
                        for qt in range(2):
                            src = pvps[:, 130 * hi + 65 * qt:
                                       130 * hi + 65 * qt + 64]
                            dst = OG[:, 2 * bl + qt, 64 * h:64 * h + 64]
                            sc = rt[:, 2 * hi + qt:2 * hi + qt + 1]
                            if qt == 0:
                                nc.scalar.mul(dst, src, sc)
                            else:
                                nc.vector.tensor_scalar_mul(dst, src, sc)

            # ---- O^T via PE transposes (bf16 stays bf16 in PSUM)
            OGT = ogtp.tile([128, 3, GT], BF16, name=f"OGT_{g}", tag="OGT")
            for ct in range(3):
                tps = t_ps.tile([128, GT], BF16, name=f"t_{g}_{ct}", tag="t")
                for tt in range(NTT):
                    nc.tensor.transpose(
                        tps[:, 128 * tt:128 * (tt + 1)],
                        OG[:, tt, 128 * ct:128 * (ct + 1)],
                        identb_sb[:],
                    )
                nc.vector.tensor_copy(OGT[:, ct, :], tps[:])

            # ---- projection [tok, C]; bias pre-loaded via K=1 matmul
            for tt in range(NTT):
                yps = mm_ps.tile([128, C], F32, name=f"y_{g}_{tt}", tag="mm")
                nc.tensor.matmul(yps[:], onecol_sb[:], beffr_sb[:],
                                 start=True, stop=False)
                for ct in range(3):
                    nc.tensor.matmul(
                        yps[:],
                        OGT[:, ct, 128 * tt:128 * (tt + 1)],
                        wp_sb[:, ct, :],
                        start=False,
                        stop=(ct == 2),
                    )
                Y = yb.tile([128, C], F32, name=f"Y_{g}_{tt}", tag="Y")
                nc.vector.tensor_copy(Y[:], yps[:])
                dma(yv[g, tt], Y[:])

        # Software pipeline one group deep: the attention stage of group
        # g-1 is emitted before the QKV stage of group g, so its small
        # LDW-heavy matmuls get priority while the big QKV matmuls fill
        # PE gaps (keeps the HAM activity monitor warm).
        prev = None
        for g in range(NG + 1):
            if prev is not None:
                attn_stage(g - 1, *prev)
            prev = qkv_stage(g) if g < NG else None


_CACHE = {}


def _build_nc():
    if "nc" in _CACHE:
        return _CACHE["nc"]
    nc = bacc.Bacc("TRN2", target_bir_lowering=False, debug=False,
                   num_devices=N_CORES)
    x_d = nc.dram_tensor("x", [TOK, C], BF16, kind="ExternalInput")
    wqk_d = nc.dram_tensor("wqk", [128, 3 * H * 128], BF16, kind="ExternalInput")
    wv_d = nc.dram_tensor("wv", [128, 3 * C], BF16, kind="ExternalInput")
    wp_d = nc.dram_tensor("wp", [128, 3 * C], BF16, kind="ExternalInput")
    bqk_d = nc.dram_tensor("bqk", [128, H], F32, kind="ExternalInput")
    beffr_d = nc.dram_tensor("beffr", [1, C], F32R, kind="ExternalInput")
    onecol_d = nc.dram_tensor("onecol", [1, 128], F32R, kind="ExternalInput")
    identb_d = nc.dram_tensor("identb", [128, 128], BF16, kind="ExternalInput")
    trimask_d = nc.dram_tensor("trimask", [128, 256], BF16, kind="ExternalInput")
    y_d = nc.dram_tensor("y", [TOK, C], F32, kind="ExternalOutput")

    with tile.TileContext(nc) as tc:
        _body(tc, x_d, wqk_d, wv_d, wp_d, bqk_d, beffr_d, onecol_d, identb_d,
              trimask_d, y_d)
    nc.compile()
    _CACHE["nc"] = nc
    return nc


def _host_inputs(x, w_attn, b_attn, w_proj, b_proj):
    """Per-core input maps; weights pre-transposed/cast on the host."""
    import ml_dtypes

    bf16 = ml_dtypes.bfloat16
    ws = np.array(w_attn, dtype=np.float32).copy()
    bs = np.array(b_attn, dtype=np.float32).copy()
    ws[:C] *= 0.125        # fold 1/sqrt(hd) into Q
    bs[:C] *= 0.125

    # wqk[k, ct, ft, m] = ws[128*ft + m, 128*ct + k]
    wqk = ws[:2 * C].reshape(H, 128, 3, 128).transpose(3, 2, 0, 1)
    wqk = np.ascontiguousarray(wqk).astype(bf16).reshape(128, 3 * H * 128)
    # wv[k, ct, n] = w_attn[2C + n, 128*ct + k]
    wv = ws[2 * C:].reshape(C, 3, 128).transpose(2, 1, 0)
    wv = np.ascontiguousarray(wv).astype(bf16).reshape(128, 3 * C)
    # wp[k, ct, n] = w_proj[n, 128*ct + k]
    wp = np.array(w_proj, np.float32).reshape(C, 3, 128).transpose(2, 1, 0)
    wp = np.ascontiguousarray(wp).astype(bf16).reshape(128, 3 * C)

    bqk = np.ascontiguousarray(bs[:2 * C].reshape(H, 128).T).astype(np.float32)
    beffr = (b_proj + w_proj @ b_attn[2 * C:]).astype(np.float32).reshape(1, C)
    onecol = np.ones((1, 128), dtype=np.float32)
    ident = np.eye(128, dtype=np.float32).astype(bf16)

    p = np.arange(128)[:, None]
    j = np.arange(128)[None, :]
    trimask = np.where(p > j, NEGBIG, 0.0).astype(np.float32).astype(bf16)
    trimask = np.concatenate([trimask, trimask], axis=1)

    common = {
        "wqk": wqk, "wv": wv, "wp": wp, "bqk": bqk, "beffr": beffr,
        "onecol": onecol, "identb": ident, "trimask": trimask,
    }
    xs = np.array(x, np.float32).reshape(N_CORES, TOK, C)
    in_maps = []
    for c in range(N_CORES):
        m = dict(common)
        m["x"] = np.ascontiguousarray(xs[c]).astype(bf16)
        in_maps.append(m)
    return in_maps


def kernel(x, w_attn, b_attn, w_proj, b_proj):
    from concourse.bass_utils import run_bass_kernel_spmd

    x = np.asarray(x, dtype=np.float32)
    w_attn = np.asarray(w_attn, dtype=np.float32)
    b_attn = np.asarray(b_attn, dtype=np.float32)
    w_proj = np.asarray(w_proj, dtype=np.float32)
    b_proj = np.asarray(b_proj, dtype=np.float32)

    nc = _build_nc()
    in_maps = _host_inputs(x, w_attn, b_attn, w_proj, b_proj)
    res = run_bass_kernel_spmd(nc, in_maps, core_ids=list(range(N_CORES)))
    y = np.stack([res.results[c]["y"] for c in range(N_CORES)])
    return y.reshape(B, T, C)


# revision 12
# speedup vs baseline: 2.8778x; 1.0082x over previous
"""Causal self-attention Trainium2 kernel (v2).

Full inputs -> full outputs. Data-parallel over batch across 8 NeuronCores
(16 batches per core), no collectives.

Per-core design (all matmul operands bf16, PSUM accumulation fp32):
  - x is cast to bf16 on the host; X^T tiles land in SBUF feature-major via
    xbar transposed DMA (no PE transposes, no eviction traffic).
  - Q^T/K^T [feat, tok] from stationary weight tiles; 1/sqrt(hd) is folded
    into the Q weights/bias on the host. Bias added during PSUM eviction
    (alternating ACT/DVE to balance engines).
  - V [tok, feat] with an interleaved ones column per head (V65) so the
    PV matmul also produces the softmax denominator Z.
  - Scores are computed causally: per (batch,head) only blocks
    (k0,q0),(k0,q1),(k1,q1); the (k1,q0) block is skipped. The causal mask
    is pre-added into PSUM for the two diagonal blocks only via N=128
    identity matmuls. Heads of a pair run in distinct 64-partition row
    groups so their K=64 score matmuls execute concurrently on the PE.
  - P^T = exp(S^T) evicted to SBUF as bf16 (single ACT op per head).
  - PV is q-major: lhsT = P^T block (stationary, 128 cols -> fast weight
    load), rhs = V65 slice (N=65). Output O[q, 64]+Z lands with queries on
    partitions, so normalization is a per-partition scale: one DVE
    reciprocal per head-pair (strided [128,4] view of Z columns) and one
    scale-by-vector eviction per (head, qtile), split across ACT and DVE.
  - O tiles are PE-transposed (bf16, stays in PSUM as bf16) and evicted to
    OGT [feat, tok]; projection runs 3 full-K=128 matmuls per token tile.
    The output bias is pre-loaded into PSUM with a K=1 ones x beff matmul;
    Y is evicted by DVE and DMAed out.
"""

import numpy as np

import concourse.bass as bass
import concourse.bacc as bacc
import concourse.mybir as mybir
import concourse.tile as tile

N_CORES = 8
B, T, C = 128, 256, 384
H, HD = 6, 64
NB = B // N_CORES          # batches per core
TOK = NB * T               # tokens per core
G = 2                      # batches per group
NG = NB // G               # groups per core
GT = G * T                 # tokens per group (512)
NTT = GT // 128            # 128-token tiles per group (4)
F32 = mybir.dt.float32
F32R = mybir.dt.float32r
BF16 = mybir.dt.bfloat16
AF = mybir.ActivationFunctionType
NEGBIG = -1.0e30


def _body(tc, x_d, wqk_d, wv_d, wp_d, bqk_d, beffr_d, onecol_d, identb_d,
          trimask_d, y_d):
    nc = tc.nc
    from contextlib import ExitStack

    ctx = ExitStack()
    with ctx:
        const = ctx.enter_context(tc.tile_pool(name="const", bufs=1))
        xt = ctx.enter_context(tc.tile_pool(name="xt", bufs=2))
        qkt = ctx.enter_context(tc.tile_pool(name="qkt", bufs=2))
        v65p = ctx.enter_context(tc.tile_pool(name="v65p", bufs=2))
        pp = ctx.enter_context(tc.tile_pool(name="pp", bufs=3))
        ogp = ctx.enter_context(tc.tile_pool(name="ogp", bufs=2))
        ogtp = ctx.enter_context(tc.tile_pool(name="ogtp", bufs=2))
        rp = ctx.enter_context(tc.tile_pool(name="rp", bufs=4))
        yb = ctx.enter_context(tc.tile_pool(name="yb", bufs=3))
        mm_ps = ctx.enter_context(tc.tile_pool(name="mm_ps", bufs=2, space="PSUM"))
        s_ps = ctx.enter_context(tc.tile_pool(name="s_ps", bufs=2, space="PSUM"))
        pv_ps = ctx.enter_context(tc.tile_pool(name="pv_ps", bufs=2, space="PSUM"))
        t_ps = ctx.enter_context(tc.tile_pool(name="t_ps", bufs=2, space="PSUM"))

        dma = nc.sync.dma_start

        wqk_sb = const.tile([128, 3, H, 128], BF16, name="wqk_sb")
        wv_sb = const.tile([128, 3, C], BF16, name="wv_sb")
        wp_sb = const.tile([128, 3, C], BF16, name="wp_sb")
        bqk_sb = const.tile([128, H], F32, name="bqk_sb")
        beffr_sb = const.tile([1, C], F32R, name="beffr_sb")
        onecol_sb = const.tile([1, 128], F32R, name="onecol_sb")
        identb_sb = const.tile([128, 128], BF16, name="identb_sb")
        trimask_sb = const.tile([128, 384], BF16, name="trimask_sb")
        ones_sb = const.tile([128, NTT * H], BF16, name="ones_sb")

        dma(wqk_sb[:], wqk_d.ap().rearrange("p (a h f) -> p a h f", a=3, h=H))
        dma(wv_sb[:], wv_d.ap().rearrange("p (a f) -> p a f", a=3))
        dma(wp_sb[:], wp_d.ap().rearrange("p (a f) -> p a f", a=3))
        dma(bqk_sb[:], bqk_d.ap())
        dma(beffr_sb[:], beffr_d.ap())
        dma(onecol_sb[:], onecol_d.ap())
        dma(identb_sb[:], identb_d.ap())
        dma(trimask_sb[:], trimask_d.ap())
        nc.vector.memset(ones_sb[:], 1.0)

        yv = y_d.ap().rearrange("(g tt p) c -> g tt p c", tt=NTT, p=128)

        def qkv_stage(g):
            # ---- X^T via xbar transposed DMA (feature-major, bf16)
            XT = xt.tile([128, 3, GT], BF16, name=f"XT_{g}", tag="XT")
            for ct in range(3):
                nc.sync.dma_start_transpose(
                    XT[:, ct, :],
                    x_d.ap()[g * GT:(g + 1) * GT, 128 * ct:128 * (ct + 1)],
                )

            # ---- Q^T / K^T  [feature, tok]; Q rows pre-scaled by 1/8
            QKT = qkt.tile([128, H, GT], BF16, name=f"QKT_{g}", tag="QKT")
            for ft in range(H):
                ps = mm_ps.tile([128, GT], F32, name=f"psqk_{g}_{ft}", tag="mm")
                for ct in range(3):
                    nc.tensor.matmul(
                        ps[:],
                        wqk_sb[:, ct, ft, :],
                        XT[:, ct, :],
                        start=(ct == 0),
                        stop=(ct == 2),
                    )
                if ft % 2 == 0:
                    nc.scalar.activation(QKT[:, ft, :], ps[:], AF.Identity,
                                         bias=bqk_sb[:, ft:ft + 1])
                else:
                    nc.vector.tensor_scalar_add(QKT[:, ft, :], ps[:],
                                                bqk_sb[:, ft:ft + 1])

            # ---- V [tok, feat] + ones column per head (V65)
            V65 = v65p.tile([128, NTT, H, 65], BF16, name=f"V65_{g}", tag="V65")
            nc.vector.tensor_copy(
                V65[:, :, :, 64],
                ones_sb[:].rearrange("p (a h) -> p a h", a=NTT),
            )
            for tt in range(NTT):
                psv = mm_ps.tile([128, C], F32, name=f"psv_{g}_{tt}", tag="mm")
                for ct in range(3):
                    nc.tensor.matmul(
                        psv[:],
                        XT[:, ct, 128 * tt:128 * (tt + 1)],
                        wv_sb[:, ct, :],
                        start=(ct == 0),
                        stop=(ct == 2),
                    )
                nc.vector.tensor_copy(
                    V65[:, tt, :, 0:64],
                    psv[:].rearrange("p (h w) -> p h w", h=H),
                )
            return QKT, V65

        def attn_stage(g, QKT, V65):
            # ---- attention per (batch, head-pair); causal blocks only
            OG = ogp.tile([128, NTT, C], BF16, name=f"OG_{g}", tag="OG")
            for bl in range(G):
                q0 = 256 * bl
                for hp in range(3):
                    pvps = pv_ps.tile([128, 260], F32, name=f"pv_{g}_{bl}_{hp}",
                                      tag="pv")
                    for hi in range(2):
                        h = 2 * hp + hi
                        r0 = 64 * (h % 2)
                        ftq, ftk = h // 2, 3 + h // 2
                        QT = QKT[r0:r0 + 64, ftq, :]
                        KT = QKT[r0:r0 + 64, ftk, :]
                        sps = s_ps.tile([128, 384], F32, name=f"s_{g}_{bl}_{h}",
                                        tag="s")
                        # One accumulation group per bank: first matmul
                        # start=True arms the whole bank pending-zero;
                        # later matmuls overwrite on first touch of their
                        # columns, accumulate after.
                        # whole-bank mask [tri | 0 | tri], one matmul
                        nc.tensor.matmul(
                            sps[:], identb_sb[:], trimask_sb[:],
                            start=True, stop=False)
                        # scores: (k0, q0+q1) then (k1,q1)
                        nc.tensor.matmul(
                            sps[:, 0:256],
                            KT[:, q0:q0 + 128], QT[:, q0:q0 + 256],
                            start=False, stop=False)
                        nc.tensor.matmul(
                            sps[:, 256:384],
                            KT[:, q0 + 128:q0 + 256], QT[:, q0 + 128:q0 + 256],
                            start=False, stop=True)
                        P = pp.tile([128, 384], BF16, name=f"P_{g}_{bl}_{h}",
                                    tag="P")
                        nc.scalar.activation(P[:], sps[:], AF.Exp)
                        # PV q-major: lhsT = P^T block, rhs = V65 slice.
                        # Single accumulation group for the shared bank.
                        c0 = 130 * hi
                        nc.tensor.matmul(
                            pvps[:, c0:c0 + 65],
                            P[:, 0:128], V65[:, 2 * bl, h, :],
                            start=(hi == 0), stop=False)
                        nc.tensor.matmul(
                            pvps[:, c0 + 65:c0 + 130],
                            P[:, 128:256], V65[:, 2 * bl, h, :],
                            start=False, stop=False)
                        nc.tensor.matmul(
                            pvps[:, c0 + 65:c0 + 130],
                            P[:, 256:384], V65[:, 2 * bl + 1, h, :],
                            start=False, stop=(hi == 1))
                    # 1/Z for both heads & qtiles: strided [128,4] view
                    rt = rp.tile([128, 4], F32, name=f"rt_{g}_{bl}_{hp}",
                                 tag="rt")
                    zview = pvps[:].rearrange("p (a c) -> p a c", c=65)[:, :, 64]
                    nc.vector.reciprocal(rt[:], zview)
                    for hi in range(2):
                        h = 2 * hp + hi
                        for qt in range(2):
                            src = pvps[:, 130 * hi + 65 * qt:
                                       130 * hi + 65 * qt + 64]
                            dst = OG[:, 2 * bl + qt, 64 * h:64 * h + 64]
                            sc = rt[:, 2 * hi + qt:2 * hi + qt + 1]
                            if qt == 0:
                                nc.scalar.mul(dst, src, sc)
                            else:
                                nc.vector.tensor_scalar_mul(dst, src, sc)

            # ---- O^T via PE transposes (bf16 stays bf16 in PSUM)
            OGT = ogtp.tile([128, 3, GT], BF16, name=f"OGT_{g}", tag="OGT")
            for ct in range(3):
                tps = t_ps.tile([128, GT], BF16, name=f"t_{g}_{ct}", tag="t")
                for tt in range(NTT):
                    nc.tensor.transpose(
                        tps[:, 128 * tt:128 * (tt + 1)],
                        OG[:, tt, 128 * ct:128 * (ct + 1)],
                        identb_sb[:],
                    )
                nc.vector.tensor_copy(OGT[:, ct, :], tps[:])

            # ---- projection [tok, C]; bias pre-loaded via K=1 matmul
            for tt in range(NTT):
                yps = mm_ps.tile([128, C], F32, name=f"y_{g}_{tt}", tag="mm")
                nc.tensor.matmul(yps[:], onecol_sb[:], beffr_sb[:],
                                 start=True, stop=False)
                for ct in range(3):
                    nc.tensor.matmul(
                        yps[:],
                        OGT[:, ct, 128 * tt:128 * (tt + 1)],
                        wp_sb[:, ct, :],
                        start=False,
                        stop=(ct == 2),
                    )
                Y = yb.tile([128, C], F32, name=f"Y_{g}_{tt}", tag="Y")
                nc.vector.tensor_copy(Y[:], yps[:])
                dma(yv[g, tt], Y[:])

        # Software pipeline one group deep: the attention stage of group
        # g-1 is emitted before the QKV stage of group g, so its small
        # LDW-heavy matmuls get priority while the big QKV matmuls fill
        # PE gaps (keeps the HAM activity monitor warm).
        prev = None
        for g in range(NG + 1):
            if prev is not None:
                attn_stage(g - 1, *prev)
            prev = qkv_stage(g) if g < NG else None


_CACHE = {}


def _build_nc():
    if "nc" in _CACHE:
        return _CACHE["nc"]
    nc = bacc.Bacc("TRN2", target_bir_lowering=False, debug=False,
                   num_devices=N_CORES)
    x_d = nc.dram_tensor("x", [TOK, C], BF16, kind="ExternalInput")
    wqk_d = nc.dram_tensor("wqk", [128, 3 * H * 128], BF16, kind="ExternalInput")
    wv_d = nc.dram_tensor("wv", [128, 3 * C], BF16, kind="ExternalInput")
    wp_d = nc.dram_tensor("wp", [128, 3 * C], BF16, kind="ExternalInput")
    bqk_d = nc.dram_tensor("bqk", [128, H], F32, kind="ExternalInput")
    beffr_d = nc.dram_tensor("beffr", [1, C], F32R, kind="ExternalInput")
    onecol_d = nc.dram_tensor("onecol", [1, 128], F32R, kind="ExternalInput")
    identb_d = nc.dram_tensor("identb", [128, 128], BF16, kind="ExternalInput")
    trimask_d = nc.dram_tensor("trimask", [128, 384], BF16, kind="ExternalInput")
    y_d = nc.dram_tensor("y", [TOK, C], F32, kind="ExternalOutput")

    with tile.TileContext(nc) as tc:
        _body(tc, x_d, wqk_d, wv_d, wp_d, bqk_d, beffr_d, onecol_d, identb_d,
              trimask_d, y_d)
    nc.compile()
    _CACHE["nc"] = nc
    return nc


def _host_inputs(x, w_attn, b_attn, w_proj, b_proj):
    """Per-core input maps; weights pre-transposed/cast on the host."""
    import ml_dtypes

    bf16 = ml_dtypes.bfloat16
    ws = np.array(w_attn, dtype=np.float32).copy()
    bs = np.array(b_attn, dtype=np.float32).copy()
    ws[:C] *= 0.125        # fold 1/sqrt(hd) into Q
    bs[:C] *= 0.125

    # wqk[k, ct, ft, m] = ws[128*ft + m, 128*ct + k]
    wqk = ws[:2 * C].reshape(H, 128, 3, 128).transpose(3, 2, 0, 1)
    wqk = np.ascontiguousarray(wqk).astype(bf16).reshape(128, 3 * H * 128)
    # wv[k, ct, n] = w_attn[2C + n, 128*ct + k]
    wv = ws[2 * C:].reshape(C, 3, 128).transpose(2, 1, 0)
    wv = np.ascontiguousarray(wv).astype(bf16).reshape(128, 3 * C)
    # wp[k, ct, n] = w_proj[n, 128*ct + k]
    wp = np.array(w_proj, np.float32).reshape(C, 3, 128).transpose(2, 1, 0)
    wp = np.ascontiguousarray(wp).astype(bf16).reshape(128, 3 * C)

    bqk = np.ascontiguousarray(bs[:2 * C].reshape(H, 128).T).astype(np.float32)
    beffr = (b_proj + w_proj @ b_attn[2 * C:]).astype(np.float32).reshape(1, C)
    onecol = np.ones((1, 128), dtype=np.float32)
    ident = np.eye(128, dtype=np.float32).astype(bf16)

    p = np.arange(128)[:, None]
    j = np.arange(128)[None, :]
    trimask = np.where(p > j, NEGBIG, 0.0).astype(np.float32).astype(bf16)
    trimask = np.concatenate([trimask, np.zeros_like(trimask), trimask], axis=1)

    common = {
        "wqk": wqk, "wv": wv, "wp": wp, "bqk": bqk, "beffr": beffr,
        "onecol": onecol, "identb": ident, "trimask": trimask,
    }
    xs = np.array(x, np.float32).reshape(N_CORES, TOK, C)
    in_maps = []
    for c in range(N_CORES):
        m = dict(common)
        m["x"] = np.ascontiguousarray(xs[c]).astype(bf16)
        in_maps.append(m)
    return in_maps


def kernel(x, w_attn, b_attn, w_proj, b_proj):
    from concourse.bass_utils import run_bass_kernel_spmd

    x = np.asarray(x, dtype=np.float32)
    w_attn = np.asarray(w_attn, dtype=np.float32)
    b_attn = np.asarray(b_attn, dtype=np.float32)
    w_proj = np.asarray(w_proj, dtype=np.float32)
    b_proj = np.asarray(b_proj, dtype=np.float32)

    nc = _build_nc()
    in_maps = _host_inputs(x, w_attn, b_attn, w_proj, b_proj)
    res = run_bass_kernel_spmd(nc, in_maps, core_ids=list(range(N_CORES)))
    y = np.stack([res.results[c]["y"] for c in range(N_CORES)])
    return y.reshape(B, T, C)


# revision 14
# speedup vs baseline: 3.5904x; 1.2476x over previous
"""Causal self-attention Trainium2 kernel (v2).

Full inputs -> full outputs. Data-parallel over batch across 8 NeuronCores
(16 batches per core), no collectives.

Per-core design (all matmul operands bf16, PSUM accumulation fp32):
  - x is cast to bf16 on the host; X^T tiles land in SBUF feature-major via
    xbar transposed DMA (no PE transposes, no eviction traffic).
  - Q^T/K^T [feat, tok] from stationary weight tiles; 1/sqrt(hd) is folded
    into the Q weights/bias on the host. Bias added during PSUM eviction
    (alternating ACT/DVE to balance engines).
  - V [tok, feat] with an interleaved ones column per head (V65) so the
    PV matmul also produces the softmax denominator Z.
  - Scores are computed causally: per (batch,head) only blocks
    (k0,q0),(k0,q1),(k1,q1); the (k1,q0) block is skipped. The causal mask
    is pre-added into PSUM for the two diagonal blocks only via N=128
    identity matmuls. Heads of a pair run in distinct 64-partition row
    groups so their K=64 score matmuls execute concurrently on the PE.
  - P^T = exp(S^T) evicted to SBUF as bf16 (single ACT op per head).
  - PV is q-major: lhsT = P^T block (stationary, 128 cols -> fast weight
    load), rhs = V65 slice (N=65). Output O[q, 64]+Z lands with queries on
    partitions, so normalization is a per-partition scale: one DVE
    reciprocal per head-pair (strided [128,4] view of Z columns) and one
    scale-by-vector eviction per (head, qtile), split across ACT and DVE.
  - O tiles are PE-transposed (bf16, stays in PSUM as bf16) and evicted to
    OGT [feat, tok]; projection runs 3 full-K=128 matmuls per token tile.
    The output bias is pre-loaded into PSUM with a K=1 ones x beff matmul;
    Y is evicted by DVE and DMAed out.
"""

import numpy as np

import concourse.bass as bass
import concourse.bacc as bacc
import concourse.mybir as mybir
import concourse.tile as tile

N_CORES = 8
B, T, C = 128, 256, 384
H, HD = 6, 64
NB = B // N_CORES          # batches per core
TOK = NB * T               # tokens per core
G = 2                      # batches per group
NG = NB // G               # groups per core
GT = G * T                 # tokens per group (512)
NTT = GT // 128            # 128-token tiles per group (4)
F32 = mybir.dt.float32
F32R = mybir.dt.float32r
BF16 = mybir.dt.bfloat16
AF = mybir.ActivationFunctionType
NEGBIG = -1.0e30


def _body(tc, x_d, wqk_d, wv_d, wp_d, bqk_d, beffr_d, onecol_d, identb_d,
          trimask_d, y_d):
    nc = tc.nc
    from contextlib import ExitStack

    ctx = ExitStack()
    with ctx:
        const = ctx.enter_context(tc.tile_pool(name="const", bufs=1))
        xt = ctx.enter_context(tc.tile_pool(name="xt", bufs=2))
        qkt = ctx.enter_context(tc.tile_pool(name="qkt", bufs=2))
        v65p = ctx.enter_context(tc.tile_pool(name="v65p", bufs=2))
        pp = ctx.enter_context(tc.tile_pool(name="pp", bufs=3))
        ogp = ctx.enter_context(tc.tile_pool(name="ogp", bufs=2))
        ogtp = ctx.enter_context(tc.tile_pool(name="ogtp", bufs=2))
        rp = ctx.enter_context(tc.tile_pool(name="rp", bufs=4))
        yb = ctx.enter_context(tc.tile_pool(name="yb", bufs=3))
        mm_ps = ctx.enter_context(tc.tile_pool(name="mm_ps", bufs=2, space="PSUM"))
        s_ps = ctx.enter_context(tc.tile_pool(name="s_ps", bufs=2, space="PSUM"))
        pv_ps = ctx.enter_context(tc.tile_pool(name="pv_ps", bufs=2, space="PSUM"))
        t_ps = ctx.enter_context(tc.tile_pool(name="t_ps", bufs=2, space="PSUM"))

        dma = nc.sync.dma_start

        wqk_sb = const.tile([128, 3, H, 128], BF16, name="wqk_sb")
        wv_sb = const.tile([128, 3, C], BF16, name="wv_sb")
        wp_sb = const.tile([128, 3, C], BF16, name="wp_sb")
        bqk_sb = const.tile([128, H], F32, name="bqk_sb")
        beffr_sb = const.tile([1, C], F32R, name="beffr_sb")
        onecol_sb = const.tile([1, 128], F32R, name="onecol_sb")
        identb_sb = const.tile([128, 128], BF16, name="identb_sb")
        trimask_sb = const.tile([128, 384], BF16, name="trimask_sb")
        ones_sb = const.tile([128, NTT * H], BF16, name="ones_sb")

        dma(wqk_sb[:], wqk_d.ap().rearrange("p (a h f) -> p a h f", a=3, h=H))
        dma(wv_sb[:], wv_d.ap().rearrange("p (a f) -> p a f", a=3))
        dma(wp_sb[:], wp_d.ap().rearrange("p (a f) -> p a f", a=3))
        dma(bqk_sb[:], bqk_d.ap())
        dma(beffr_sb[:], beffr_d.ap())
        dma(onecol_sb[:], onecol_d.ap())
        dma(identb_sb[:], identb_d.ap())
        dma(trimask_sb[:], trimask_d.ap())
        nc.vector.memset(ones_sb[:], 1.0)

        yv = y_d.ap().rearrange("(g tt p) c -> g tt p c", tt=NTT, p=128)

        def xt_dma(g):
            # ---- X^T via xbar transposed DMA (feature-major, bf16)
            XT = xt.tile([128, 3, GT], BF16, name=f"XT_{g}", tag="XT")
            for ct in range(3):
                nc.sync.dma_start_transpose(
                    XT[:, ct, :],
                    x_d.ap()[g * GT:(g + 1) * GT, 128 * ct:128 * (ct + 1)],
                )
            return XT

        def qkt_ft(g, XT, QKT, ft):
            # ---- Q^T / K^T  [feature, tok]; Q rows pre-scaled by 1/8
            ps = mm_ps.tile([128, GT], F32, name=f"psqk_{g}_{ft}", tag="mm")
            for ct in range(3):
                nc.tensor.matmul(
                    ps[:],
                    wqk_sb[:, ct, ft, :],
                    XT[:, ct, :],
                    start=(ct == 0),
                    stop=(ct == 2),
                )
            if ft % 2 == 0:
                nc.scalar.activation(QKT[:, ft, :], ps[:], AF.Identity,
                                     bias=bqk_sb[:, ft:ft + 1])
            else:
                nc.vector.tensor_scalar_add(QKT[:, ft, :], ps[:],
                                            bqk_sb[:, ft:ft + 1])

        def v_tt(g, XT, V65, tt):
            # ---- V [tok, feat] + ones column per head (V65)
            psv = mm_ps.tile([128, C], F32, name=f"psv_{g}_{tt}", tag="mm")
            for ct in range(3):
                nc.tensor.matmul(
                    psv[:],
                    XT[:, ct, 128 * tt:128 * (tt + 1)],
                    wv_sb[:, ct, :],
                    start=(ct == 0),
                    stop=(ct == 2),
                )
            nc.vector.tensor_copy(
                V65[:, tt, :, 0:64],
                psv[:].rearrange("p (h w) -> p h w", h=H),
            )

        def attn_pair(g, QKT, V65, OG, bl, hp):
            # ---- attention for one (batch, head-pair); causal blocks only
            if True:
                q0 = 256 * bl
                if True:
                    pvps = pv_ps.tile([128, 260], F32, name=f"pv_{g}_{bl}_{hp}",
                                      tag="pv")
                    for hi in range(2):
                        h = 2 * hp + hi
                        r0 = 64 * (h % 2)
                        ftq, ftk = h // 2, 3 + h // 2
                        QT = QKT[r0:r0 + 64, ftq, :]
                        KT = QKT[r0:r0 + 64, ftk, :]
                        sps = s_ps.tile([128, 384], F32, name=f"s_{g}_{bl}_{h}",
                                        tag="s")
                        # One accumulation group per bank: first matmul
                        # start=True arms the whole bank pending-zero;
                        # later matmuls overwrite on first touch of their
                        # columns, accumulate after.
                        # whole-bank mask [tri | 0 | tri], one matmul
                        nc.tensor.matmul(
                            sps[:], identb_sb[:], trimask_sb[:],
                            start=True, stop=False)
                        # scores: (k0, q0+q1) then (k1,q1)
                        nc.tensor.matmul(
                            sps[:, 0:256],
                            KT[:, q0:q0 + 128], QT[:, q0:q0 + 256],
                            start=False, stop=False)
                        nc.tensor.matmul(
                            sps[:, 256:384],
                            KT[:, q0 + 128:q0 + 256], QT[:, q0 + 128:q0 + 256],
                            start=False, stop=True)
                        P = pp.tile([128, 384], BF16, name=f"P_{g}_{bl}_{h}",
                                    tag="P")
                        nc.scalar.activation(P[:], sps[:], AF.Exp)
                        # PV q-major: lhsT = P^T block, rhs = V65 slice.
                        # Single accumulation group for the shared bank.
                        c0 = 130 * hi
                        nc.tensor.matmul(
                            pvps[:, c0:c0 + 65],
                            P[:, 0:128], V65[:, 2 * bl, h, :],
                            start=(hi == 0), stop=False)
                        nc.tensor.matmul(
                            pvps[:, c0 + 65:c0 + 130],
                            P[:, 128:256], V65[:, 2 * bl, h, :],
                            start=False, stop=False)
                        nc.tensor.matmul(
                            pvps[:, c0 + 65:c0 + 130],
                            P[:, 256:384], V65[:, 2 * bl + 1, h, :],
                            start=False, stop=(hi == 1))
                    # 1/Z for both heads & qtiles: strided [128,4] view
                    rt = rp.tile([128, 4], F32, name=f"rt_{g}_{bl}_{hp}",
                                 tag="rt")
                    zview = pvps[:].rearrange("p (a c) -> p a c", c=65)[:, :, 64]
                    nc.vector.reciprocal(rt[:], zview)
                    for hi in range(2):
                        h = 2 * hp + hi
                        for qt in range(2):
                            src = pvps[:, 130 * hi + 65 * qt:
                                       130 * hi + 65 * qt + 64]
                            dst = OG[:, 2 * bl + qt, 64 * h:64 * h + 64]
                            sc = rt[:, 2 * hi + qt:2 * hi + qt + 1]
                            if qt == 0:
                                nc.scalar.mul(dst, src, sc)
                            else:
                                nc.vector.tensor_scalar_mul(dst, src, sc)

        def ogt_ct(g, OG, OGT, ct):
            # ---- O^T via PE transposes (bf16 stays bf16 in PSUM)
            tps = t_ps.tile([128, GT], BF16, name=f"t_{g}_{ct}", tag="t")
            for tt in range(NTT):
                nc.tensor.transpose(
                    tps[:, 128 * tt:128 * (tt + 1)],
                    OG[:, tt, 128 * ct:128 * (ct + 1)],
                    identb_sb[:],
                )
            nc.vector.tensor_copy(OGT[:, ct, :], tps[:])

        def proj_tt(g, OGT, tt):
            # ---- projection [tok, C]; bias pre-loaded via K=1 matmul
            yps = mm_ps.tile([128, C], F32, name=f"y_{g}_{tt}", tag="mm")
            nc.tensor.matmul(yps[:], onecol_sb[:], beffr_sb[:],
                             start=True, stop=False)
            for ct in range(3):
                nc.tensor.matmul(
                    yps[:],
                    OGT[:, ct, 128 * tt:128 * (tt + 1)],
                    wp_sb[:, ct, :],
                    start=False,
                    stop=(ct == 2),
                )
            Y = yb.tile([128, C], F32, name=f"Y_{g}_{tt}", tag="Y")
            nc.vector.tensor_copy(Y[:], yps[:])
            dma(yv[g, tt], Y[:])

        # Software pipeline one group deep with fine-grained
        # interleaving: each LDW-heavy attention pair of group g-1 is
        # followed by a big QKV matmul of group g so the PE array duty
        # stays above the HAM activity threshold (avoids re-throttle).
        prev = None
        for g in range(NG + 1):
            XT = xt_dma(g) if g < NG else None
            QKT = qkt.tile([128, H, GT], BF16, name=f"QKT_{g}", tag="QKT") \
                if g < NG else None
            V65 = v65p.tile([128, NTT, H, 65], BF16, name=f"V65_{g}",
                            tag="V65") if g < NG else None
            if V65 is not None:
                nc.vector.tensor_copy(
                    V65[:, :, :, 64],
                    ones_sb[:].rearrange("p (a h) -> p a h", a=NTT),
                )
            OG = ogp.tile([128, NTT, C], BF16, name=f"OG_{g-1}", tag="OG") \
                if prev is not None else None
            OGT = ogtp.tile([128, 3, GT], BF16, name=f"OGT_{g-1}", tag="OGT") \
                if prev is not None else None

            pairs = [(bl, hp) for bl in range(G) for hp in range(3)]
            for i in range(6):
                if prev is not None:
                    attn_pair(g - 1, prev[0], prev[1], OG, *pairs[i])
                if g < NG:
                    qkt_ft(g, XT, QKT, i)
            tail = []
            if prev is not None:
                tail += [lambda c=c: ogt_ct(g - 1, OG, OGT, c) for c in range(3)]
                tail += [lambda t=t: proj_tt(g - 1, OGT, t) for t in range(NTT)]
            fill = [lambda t=t: v_tt(g, XT, V65, t) for t in range(NTT)] \
                if g < NG else []
            # round-robin the tails with the V fills
            out = []
            while tail or fill:
                if tail:
                    out.append(tail.pop(0))
                if fill:
                    out.append(fill.pop(0))
            for f in out:
                f()
            prev = (QKT, V65) if g < NG else None


_CACHE = {}


def _build_nc():
    if "nc" in _CACHE:
        return _CACHE["nc"]
    nc = bacc.Bacc("TRN2", target_bir_lowering=False, debug=False,
                   num_devices=N_CORES)
    x_d = nc.dram_tensor("x", [TOK, C], BF16, kind="ExternalInput")
    wqk_d = nc.dram_tensor("wqk", [128, 3 * H * 128], BF16, kind="ExternalInput")
    wv_d = nc.dram_tensor("wv", [128, 3 * C], BF16, kind="ExternalInput")
    wp_d = nc.dram_tensor("wp", [128, 3 * C], BF16, kind="ExternalInput")
    bqk_d = nc.dram_tensor("bqk", [128, H], F32, kind="ExternalInput")
    beffr_d = nc.dram_tensor("beffr", [1, C], F32R, kind="ExternalInput")
    onecol_d = nc.dram_tensor("onecol", [1, 128], F32R, kind="ExternalInput")
    identb_d = nc.dram_tensor("identb", [128, 128], BF16, kind="ExternalInput")
    trimask_d = nc.dram_tensor("trimask", [128, 384], BF16, kind="ExternalInput")
    y_d = nc.dram_tensor("y", [TOK, C], F32, kind="ExternalOutput")

    with tile.TileContext(nc) as tc:
        _body(tc, x_d, wqk_d, wv_d, wp_d, bqk_d, beffr_d, onecol_d, identb_d,
              trimask_d, y_d)
    nc.compile()
    _CACHE["nc"] = nc
    return nc


def _host_inputs(x, w_attn, b_attn, w_proj, b_proj):
    """Per-core input maps; weights pre-transposed/cast on the host."""
    import ml_dtypes

    bf16 = ml_dtypes.bfloat16
    ws = np.array(w_attn, dtype=np.float32).copy()
    bs = np.array(b_attn, dtype=np.float32).copy()
    ws[:C] *= 0.125        # fold 1/sqrt(hd) into Q
    bs[:C] *= 0.125

    # wqk[k, ct, ft, m] = ws[128*ft + m, 128*ct + k]
    wqk = ws[:2 * C].reshape(H, 128, 3, 128).transpose(3, 2, 0, 1)
    wqk = np.ascontiguousarray(wqk).astype(bf16).reshape(128, 3 * H * 128)
    # wv[k, ct, n] = w_attn[2C + n, 128*ct + k]
    wv = ws[2 * C:].reshape(C, 3, 128).transpose(2, 1, 0)
    wv = np.ascontiguousarray(wv).astype(bf16).reshape(128, 3 * C)
    # wp[k, ct, n] = w_proj[n, 128*ct + k]
    wp = np.array(w_proj, np.float32).reshape(C, 3, 128).transpose(2, 1, 0)
    wp = np.ascontiguousarray(wp).astype(bf16).reshape(128, 3 * C)

    bqk = np.ascontiguousarray(bs[:2 * C].reshape(H, 128).T).astype(np.float32)
    beffr = (b_proj + w_proj @ b_attn[2 * C:]).astype(np.float32).reshape(1, C)
    onecol = np.ones((1, 128), dtype=np.float32)
    ident = np.eye(128, dtype=np.float32).astype(bf16)

    p = np.arange(128)[:, None]
    j = np.arange(128)[None, :]
    trimask = np.where(p > j, NEGBIG, 0.0).astype(np.float32).astype(bf16)
    trimask = np.concatenate([trimask, np.zeros_like(trimask), trimask], axis=1)

    common = {
        "wqk": wqk, "wv": wv, "wp": wp, "bqk": bqk, "beffr": beffr,
        "onecol": onecol, "identb": ident, "trimask": trimask,
    }
    xs = np.array(x, np.float32).reshape(N_CORES, TOK, C)
    in_maps = []
    for c in range(N_CORES):
        m = dict(common)
        m["x"] = np.ascontiguousarray(xs[c]).astype(bf16)
        in_maps.append(m)
    return in_maps


def kernel(x, w_attn, b_attn, w_proj, b_proj):
    from concourse.bass_utils import run_bass_kernel_spmd

    x = np.asarray(x, dtype=np.float32)
    w_attn = np.asarray(w_attn, dtype=np.float32)
    b_attn = np.asarray(b_attn, dtype=np.float32)
    w_proj = np.asarray(w_proj, dtype=np.float32)
    b_proj = np.asarray(b_proj, dtype=np.float32)

    nc = _build_nc()
    in_maps = _host_inputs(x, w_attn, b_attn, w_proj, b_proj)
    res = run_bass_kernel_spmd(nc, in_maps, core_ids=list(range(N_CORES)))
    y = np.stack([res.results[c]["y"] for c in range(N_CORES)])
    return y.reshape(B, T, C)


# revision 15
# speedup vs baseline: 3.7579x; 1.0467x over previous
"""Causal self-attention Trainium2 kernel (v2).

Full inputs -> full outputs. Data-parallel over batch across 8 NeuronCores
(16 batches per core), no collectives.

Per-core design (all matmul operands bf16, PSUM accumulation fp32):
  - x is cast to bf16 on the host; X^T tiles land in SBUF feature-major via
    xbar transposed DMA (no PE transposes, no eviction traffic).
  - Q^T/K^T [feat, tok] from stationary weight tiles; 1/sqrt(hd) is folded
    into the Q weights/bias on the host. Bias added during PSUM eviction
    (alternating ACT/DVE to balance engines).
  - V [tok, feat] with an interleaved ones column per head (V65) so the
    PV matmul also produces the softmax denominator Z.
  - Scores are computed causally: per (batch,head) only blocks
    (k0,q0),(k0,q1),(k1,q1); the (k1,q0) block is skipped. The causal mask
    is pre-added into PSUM for the two diagonal blocks only via N=128
    identity matmuls. Heads of a pair run in distinct 64-partition row
    groups so their K=64 score matmuls execute concurrently on the PE.
  - P^T = exp(S^T) evicted to SBUF as bf16 (single ACT op per head).
  - PV is q-major: lhsT = P^T block (stationary, 128 cols -> fast weight
    load), rhs = V65 slice (N=65). Output O[q, 64]+Z lands with queries on
    partitions, so normalization is a per-partition scale: one DVE
    reciprocal per head-pair (strided [128,4] view of Z columns) and one
    scale-by-vector eviction per (head, qtile), split across ACT and DVE.
  - O tiles are PE-transposed (bf16, stays in PSUM as bf16) and evicted to
    OGT [feat, tok]; projection runs 3 full-K=128 matmuls per token tile.
    The output bias is pre-loaded into PSUM with a K=1 ones x beff matmul;
    Y is evicted by DVE and DMAed out.
"""

import numpy as np

import concourse.bass as bass
import concourse.bacc as bacc
import concourse.mybir as mybir
import concourse.tile as tile

N_CORES = 8
B, T, C = 128, 256, 384
H, HD = 6, 64
NB = B // N_CORES          # batches per core
TOK = NB * T               # tokens per core
G = 2                      # batches per group
NG = NB // G               # groups per core
GT = G * T                 # tokens per group (512)
NTT = GT // 128            # 128-token tiles per group (4)
F32 = mybir.dt.float32
F32R = mybir.dt.float32r
BF16 = mybir.dt.bfloat16
AF = mybir.ActivationFunctionType
NEGBIG = -1.0e30


def _body(tc, x_d, wqk_d, wv_d, wp_d, bqk_d, beffr_d, onecol_d, identb_d,
          trimask_d, y_d):
    nc = tc.nc
    from contextlib import ExitStack

    ctx = ExitStack()
    with ctx:
        const = ctx.enter_context(tc.tile_pool(name="const", bufs=1))
        xt = ctx.enter_context(tc.tile_pool(name="xt", bufs=2))
        qkt = ctx.enter_context(tc.tile_pool(name="qkt", bufs=2))
        v65p = ctx.enter_context(tc.tile_pool(name="v65p", bufs=2))
        pp = ctx.enter_context(tc.tile_pool(name="pp", bufs=4))
        ogp = ctx.enter_context(tc.tile_pool(name="ogp", bufs=2))
        ogtp = ctx.enter_context(tc.tile_pool(name="ogtp", bufs=2))
        rp = ctx.enter_context(tc.tile_pool(name="rp", bufs=4))
        yb = ctx.enter_context(tc.tile_pool(name="yb", bufs=3))
        mm_ps = ctx.enter_context(tc.tile_pool(name="mm_ps", bufs=2, space="PSUM"))
        s_ps = ctx.enter_context(tc.tile_pool(name="s_ps", bufs=3, space="PSUM"))
        pv_ps = ctx.enter_context(tc.tile_pool(name="pv_ps", bufs=3, space="PSUM"))

        dma = nc.sync.dma_start

        wqk_sb = const.tile([128, 3, H, 128], BF16, name="wqk_sb")
        wv_sb = const.tile([128, 3, C], BF16, name="wv_sb")
        wp_sb = const.tile([128, 3, C], BF16, name="wp_sb")
        bqk_sb = const.tile([128, H], F32, name="bqk_sb")
        beffr_sb = const.tile([1, C], F32R, name="beffr_sb")
        onecol_sb = const.tile([1, 128], F32R, name="onecol_sb")
        identb_sb = const.tile([128, 128], BF16, name="identb_sb")
        trimask_sb = const.tile([128, 384], BF16, name="trimask_sb")
        ones_sb = const.tile([128, NTT * H], BF16, name="ones_sb")

        dma(wqk_sb[:], wqk_d.ap().rearrange("p (a h f) -> p a h f", a=3, h=H))
        dma(wv_sb[:], wv_d.ap().rearrange("p (a f) -> p a f", a=3))
        dma(wp_sb[:], wp_d.ap().rearrange("p (a f) -> p a f", a=3))
        dma(bqk_sb[:], bqk_d.ap())
        dma(beffr_sb[:], beffr_d.ap())
        dma(onecol_sb[:], onecol_d.ap())
        dma(identb_sb[:], identb_d.ap())
        dma(trimask_sb[:], trimask_d.ap())
        nc.vector.memset(ones_sb[:], 1.0)

        yv = y_d.ap().rearrange("(g tt p) c -> g tt p c", tt=NTT, p=128)

        def xt_dma(g):
            # ---- X^T via xbar transposed DMA (feature-major, bf16)
            XT = xt.tile([128, 3, GT], BF16, name=f"XT_{g}", tag="XT")
            for ct in range(3):
                nc.sync.dma_start_transpose(
                    XT[:, ct, :],
                    x_d.ap()[g * GT:(g + 1) * GT, 128 * ct:128 * (ct + 1)],
                )
            return XT

        def qkt_ft(g, XT, QKT, ft):
            # ---- Q^T / K^T  [feature, tok]; Q rows pre-scaled by 1/8
            ps = mm_ps.tile([128, GT], F32, name=f"psqk_{g}_{ft}", tag="mm")
            for ct in range(3):
                nc.tensor.matmul(
                    ps[:],
                    wqk_sb[:, ct, ft, :],
                    XT[:, ct, :],
                    start=(ct == 0),
                    stop=(ct == 2),
                )
            if ft % 2 == 0:
                nc.scalar.activation(QKT[:, ft, :], ps[:], AF.Identity,
                                     bias=bqk_sb[:, ft:ft + 1])
            else:
                nc.vector.tensor_scalar_add(QKT[:, ft, :], ps[:],
                                            bqk_sb[:, ft:ft + 1])

        def v_tt(g, XT, V65, tt):
            # ---- V [tok, feat] + ones column per head (V65)
            psv = mm_ps.tile([128, C], F32, name=f"psv_{g}_{tt}", tag="mm")
            for ct in range(3):
                nc.tensor.matmul(
                    psv[:],
                    XT[:, ct, 128 * tt:128 * (tt + 1)],
                    wv_sb[:, ct, :],
                    start=(ct == 0),
                    stop=(ct == 2),
                )
            nc.vector.tensor_copy(
                V65[:, tt, :, 0:64],
                psv[:].rearrange("p (h w) -> p h w", h=H),
            )

        def attn_pair(g, QKT, V65, OG, bl, hp):
            # ---- attention for one (batch, head-pair); causal blocks only
            if True:
                q0 = 256 * bl
                if True:
                    pvps = pv_ps.tile([128, 260], F32, name=f"pv_{g}_{bl}_{hp}",
                                      tag="pv")
                    for hi in range(2):
                        h = 2 * hp + hi
                        r0 = 64 * (h % 2)
                        ftq, ftk = h // 2, 3 + h // 2
                        QT = QKT[r0:r0 + 64, ftq, :]
                        KT = QKT[r0:r0 + 64, ftk, :]
                        sps = s_ps.tile([128, 384], F32, name=f"s_{g}_{bl}_{h}",
                                        tag="s")
                        # One accumulation group per bank: first matmul
                        # start=True arms the whole bank pending-zero;
                        # later matmuls overwrite on first touch of their
                        # columns, accumulate after.
                        # whole-bank mask [tri | 0 | tri], one matmul
                        nc.tensor.matmul(
                            sps[:], identb_sb[:], trimask_sb[:],
                            start=True, stop=False)
                        # scores: (k0, q0+q1) then (k1,q1)
                        nc.tensor.matmul(
                            sps[:, 0:256],
                            KT[:, q0:q0 + 128], QT[:, q0:q0 + 256],
                            start=False, stop=False)
                        nc.tensor.matmul(
                            sps[:, 256:384],
                            KT[:, q0 + 128:q0 + 256], QT[:, q0 + 128:q0 + 256],
                            start=False, stop=True)
                        P = pp.tile([128, 384], BF16, name=f"P_{g}_{bl}_{h}",
                                    tag="P")
                        nc.scalar.activation(P[:], sps[:], AF.Exp)
                        # PV q-major: lhsT = P^T block, rhs = V65 slice.
                        # Single accumulation group for the shared bank.
                        c0 = 130 * hi
                        nc.tensor.matmul(
                            pvps[:, c0:c0 + 65],
                            P[:, 0:128], V65[:, 2 * bl, h, :],
                            start=(hi == 0), stop=False)
                        nc.tensor.matmul(
                            pvps[:, c0 + 65:c0 + 130],
                            P[:, 128:256], V65[:, 2 * bl, h, :],
                            start=False, stop=False)
                        nc.tensor.matmul(
                            pvps[:, c0 + 65:c0 + 130],
                            P[:, 256:384], V65[:, 2 * bl + 1, h, :],
                            start=False, stop=(hi == 1))
                    # 1/Z for both heads & qtiles: strided [128,4] view
                    rt = rp.tile([128, 4], F32, name=f"rt_{g}_{bl}_{hp}",
                                 tag="rt")
                    zview = pvps[:].rearrange("p (a c) -> p a c", c=65)[:, :, 64]
                    nc.vector.reciprocal(rt[:], zview)
                    for hi in range(2):
                        h = 2 * hp + hi
                        for qt in range(2):
                            src = pvps[:, 130 * hi + 65 * qt:
                                       130 * hi + 65 * qt + 64]
                            dst = OG[:, 2 * bl + qt, 64 * h:64 * h + 64]
                            sc = rt[:, 2 * hi + qt:2 * hi + qt + 1]
                            if qt == 0:
                                nc.scalar.mul(dst, src, sc)
                            else:
                                nc.vector.tensor_scalar_mul(dst, src, sc)

        def ogt_ct(g, OG, OGT, ct):
            # ---- O^T via PE transposes (bf16 stays bf16 in PSUM)
            tps = mm_ps.tile([128, GT], BF16, name=f"t_{g}_{ct}", tag="mm")
            for tt in range(NTT):
                nc.tensor.transpose(
                    tps[:, 128 * tt:128 * (tt + 1)],
                    OG[:, tt, 128 * ct:128 * (ct + 1)],
                    identb_sb[:],
                )
            nc.vector.tensor_copy(OGT[:, ct, :], tps[:])

        def proj_tt(g, OGT, tt):
            # ---- projection [tok, C]; bias pre-loaded via K=1 matmul
            yps = mm_ps.tile([128, C], F32, name=f"y_{g}_{tt}", tag="mm")
            nc.tensor.matmul(yps[:], onecol_sb[:], beffr_sb[:],
                             start=True, stop=False)
            for ct in range(3):
                nc.tensor.matmul(
                    yps[:],
                    OGT[:, ct, 128 * tt:128 * (tt + 1)],
                    wp_sb[:, ct, :],
                    start=False,
                    stop=(ct == 2),
                )
            Y = yb.tile([128, C], F32, name=f"Y_{g}_{tt}", tag="Y")
            nc.vector.tensor_copy(Y[:], yps[:])
            dma(yv[g, tt], Y[:])

        # Software pipeline one group deep with fine-grained
        # interleaving: each LDW-heavy attention pair of group g-1 is
        # followed by a big QKV matmul of group g so the PE array duty
        # stays above the HAM activity threshold (avoids re-throttle).
        prev = None
        for g in range(NG + 1):
            XT = xt_dma(g) if g < NG else None
            QKT = qkt.tile([128, H, GT], BF16, name=f"QKT_{g}", tag="QKT") \
                if g < NG else None
            V65 = v65p.tile([128, NTT, H, 65], BF16, name=f"V65_{g}",
                            tag="V65") if g < NG else None
            if V65 is not None:
                nc.vector.tensor_copy(
                    V65[:, :, :, 64],
                    ones_sb[:].rearrange("p (a h) -> p a h", a=NTT),
                )
            OG = ogp.tile([128, NTT, C], BF16, name=f"OG_{g-1}", tag="OG") \
                if prev is not None else None
            OGT = ogtp.tile([128, 3, GT], BF16, name=f"OGT_{g-1}", tag="OGT") \
                if prev is not None else None

            pairs = [(bl, hp) for bl in range(G) for hp in range(3)]
            for i in range(6):
                if prev is not None:
                    attn_pair(g - 1, prev[0], prev[1], OG, *pairs[i])
                if g < NG:
                    qkt_ft(g, XT, QKT, i)
            tail = []
            if prev is not None:
                tail += [lambda c=c: ogt_ct(g - 1, OG, OGT, c) for c in range(3)]
                tail += [lambda t=t: proj_tt(g - 1, OGT, t) for t in range(NTT)]
            fill = [lambda t=t: v_tt(g, XT, V65, t) for t in range(NTT)] \
                if g < NG else []
            # round-robin the tails with the V fills
            out = []
            while tail or fill:
                if tail:
                    out.append(tail.pop(0))
                if fill:
                    out.append(fill.pop(0))
            for f in out:
                f()
            prev = (QKT, V65) if g < NG else None


_CACHE = {}


def _build_nc():
    if "nc" in _CACHE:
        return _CACHE["nc"]
    nc = bacc.Bacc("TRN2", target_bir_lowering=False, debug=False,
                   num_devices=N_CORES)
    x_d = nc.dram_tensor("x", [TOK, C], BF16, kind="ExternalInput")
    wqk_d = nc.dram_tensor("wqk", [128, 3 * H * 128], BF16, kind="ExternalInput")
    wv_d = nc.dram_tensor("wv", [128, 3 * C], BF16, kind="ExternalInput")
    wp_d = nc.dram_tensor("wp", [128, 3 * C], BF16, kind="ExternalInput")
    bqk_d = nc.dram_tensor("bqk", [128, H], F32, kind="ExternalInput")
    beffr_d = nc.dram_tensor("beffr", [1, C], F32R, kind="ExternalInput")
    onecol_d = nc.dram_tensor("onecol", [1, 128], F32R, kind="ExternalInput")
    identb_d = nc.dram_tensor("identb", [128, 128], BF16, kind="ExternalInput")
    trimask_d = nc.dram_tensor("trimask", [128, 384], BF16, kind="ExternalInput")
    y_d = nc.dram_tensor("y", [TOK, C], F32, kind="ExternalOutput")

    with tile.TileContext(nc) as tc:
        _body(tc, x_d, wqk_d, wv_d, wp_d, bqk_d, beffr_d, onecol_d, identb_d,
              trimask_d, y_d)
    nc.compile()
    _CACHE["nc"] = nc
    return nc


def _host_inputs(x, w_attn, b_attn, w_proj, b_proj):
    """Per-core input maps; weights pre-transposed/cast on the host."""
    import ml_dtypes

    bf16 = ml_dtypes.bfloat16
    ws = np.array(w_attn, dtype=np.float32).copy()
    bs = np.array(b_attn, dtype=np.float32).copy()
    ws[:C] *= 0.125        # fold 1/sqrt(hd) into Q
    bs[:C] *= 0.125

    # wqk[k, ct, ft, m] = ws[128*ft + m, 128*ct + k]
    wqk = ws[:2 * C].reshape(H, 128, 3, 128).transpose(3, 2, 0, 1)
    wqk = np.ascontiguousarray(wqk).astype(bf16).reshape(128, 3 * H * 128)
    # wv[k, ct, n] = w_attn[2C + n, 128*ct + k]
    wv = ws[2 * C:].reshape(C, 3, 128).transpose(2, 1, 0)
    wv = np.ascontiguousarray(wv).astype(bf16).reshape(128, 3 * C)
    # wp[k, ct, n] = w_proj[n, 128*ct + k]
    wp = np.array(w_proj, np.float32).reshape(C, 3, 128).transpose(2, 1, 0)
    wp = np.ascontiguousarray(wp).astype(bf16).reshape(128, 3 * C)

    bqk = np.ascontiguousarray(bs[:2 * C].reshape(H, 128).T).astype(np.float32)
    beffr = (b_proj + w_proj @ b_attn[2 * C:]).astype(np.float32).reshape(1, C)
    onecol = np.ones((1, 128), dtype=np.float32)
    ident = np.eye(128, dtype=np.float32).astype(bf16)

    p = np.arange(128)[:, None]
    j = np.arange(128)[None, :]
    trimask = np.where(p > j, NEGBIG, 0.0).astype(np.float32).astype(bf16)
    trimask = np.concatenate([trimask, np.zeros_like(trimask), trimask], axis=1)

    common = {
        "wqk": wqk, "wv": wv, "wp": wp, "bqk": bqk, "beffr": beffr,
        "onecol": onecol, "identb": ident, "trimask": trimask,
    }
    xs = np.array(x, np.float32).reshape(N_CORES, TOK, C)
    in_maps = []
    for c in range(N_CORES):
        m = dict(common)
        m["x"] = np.ascontiguousarray(xs[c]).astype(bf16)
        in_maps.append(m)
    return in_maps


def kernel(x, w_attn, b_attn, w_proj, b_proj):
    from concourse.bass_utils import run_bass_kernel_spmd

    x = np.asarray(x, dtype=np.float32)
    w_attn = np.asarray(w_attn, dtype=np.float32)
    b_attn = np.asarray(b_attn, dtype=np.float32)
    w_proj = np.asarray(w_proj, dtype=np.float32)
    b_proj = np.asarray(b_proj, dtype=np.float32)

    nc = _build_nc()
    in_maps = _host_inputs(x, w_attn, b_attn, w_proj, b_proj)
    res = run_bass_kernel_spmd(nc, in_maps, core_ids=list(range(N_CORES)))
    y = np.stack([res.results[c]["y"] for c in range(N_CORES)])
    return y.reshape(B, T, C)
